# revision 11
# baseline (speedup 1.0000x reference)
"""Trainium2 Bass kernel for nn_Attention (B=4, N=2048, dim=1024, 16 heads).

Sharding: each of the 8 cores handles one (batch, head-group) pair —
batch b = core//2, head-group g = core%2 (8 heads each). Per core:
  qkv part  : Q^T,K^T = W_{q,k}[g] @ x_b^T (d-major), V natural
  attention : S^T = K^T-slice.T @ Q^T-slice  (k on partitions, the two
              heads of a pair row-tiled onto PE rows 0-63 / 64-127),
              P^T = exp(S^T * scale)  (no max subtraction: logits ~ N(0,1)),
              O^T[d,q] = V.T @ P^T per head, the two heads of a pair
              COLUMN-tiled onto PE col groups 0-63 / 64-127 so both run
              concurrently in the array (one 512-col pass per k-chunk).
  denoms    : softmax1 denominators via ones-stationary matmuls, four
              M=32 column-tiles per slot covering two heads x two
              k-chunks (quarter the moving cost of a fused ones-column).
  normalize : recip = 1/(1+den) on DVE, partition-broadcast via DMA,
              multiply O^T(PSUM) x recip -> H^T (bf16 SBUF).
  proj      : OUT^T = Wp[g]-slices @ H^T   (partial over head channels)
Host side: per batch, out[b] = (OUT^T_{2b} + OUT^T_{2b+1}).T + proj_b.

The exp stream on the Scalar engine (256 x [128,1024] activations)
paces the kernel; all PE work besides scores is woven between exp units
in fine-grained pieces so no unit overloads the PE past the ACT period.
"""

import numpy as np
import ml_dtypes
from contextlib import ExitStack

import concourse.bass as bass
import concourse.tile as tile
from concourse import mybir
from concourse.bass_utils import run_bass_kernel_spmd

BF16 = mybir.dt.bfloat16
F32 = mybir.dt.float32
AF = mybir.ActivationFunctionType
NPBF16 = ml_dtypes.bfloat16

N_CORES = 8
B = 4
N = 2048          # tokens per batch
C = 1024          # model dim
NH = 8            # heads per core
HD = 64           # head dim
DQ = NH * HD      # q/k/v dims per core (512)
SCALE = HD ** -0.5
CC = C // 128     # contraction chunks (8)
QB = N // 512     # q blocks of 512 (4)
KC = N // 128     # k chunks of 128 (16)
HC = DQ // 128    # head pairs (4)
OT = C // 128     # output row tiles (8)
_MAX_WAITS = 1


def _split_excess_waits(nc):
    """This walrus build rejects >1 semaphore wait per instruction
    ("Too many sync wait commands"); move the excess onto NOPs inserted
    immediately before the offending instruction on the same engine."""
    n_new = 0
    for f in nc.m.functions:
        for bb in f.blocks:
            insts = bb.instructions
            i = 0
            while i < len(insts):
                inst = insts[i]
                si = inst.sync_info
                if si is not None and si.on_wait and len(si.on_wait) > _MAX_WAITS:
                    waits = list(si.on_wait)
                    keep, rest = waits[:_MAX_WAITS], waits[_MAX_WAITS:]
                    nops = []
                    while rest:
                        chunk, rest = rest[:_MAX_WAITS], rest[_MAX_WAITS:]
                        nop = mybir.InstNoOp(
                            name=f"wait-split-{n_new}", ins=[], outs=[])
                        n_new += 1
                        nop.engine = inst.engine
                        nop.sync_info = mybir.SyncInfo(on_wait=chunk, on_update=[])
                        nops.append(nop)
                    inst.sync_info = mybir.SyncInfo(
                        on_wait=keep, on_update=list(si.on_update or []))
                    for j, nop in enumerate(nops):
                        insts.insert(i + j, nop)
                    i += len(nops)
                i += 1
    return n_new


def _build(ctx: ExitStack, tc: tile.TileContext, xT, wqT, wkT, wvT, wpT, outP):
    nc = tc.nc

    persist = ctx.enter_context(tc.tile_pool(name="persist", bufs=1))
    p_pool = ctx.enter_context(tc.tile_pool(name="p", bufs=6))
    stg_pool = ctx.enter_context(tc.tile_pool(name="stg", bufs=3))
    s_pool = ctx.enter_context(tc.tile_pool(name="dsb", bufs=2))
    u_pool = ctx.enter_context(tc.tile_pool(name="du", bufs=2))
    rb_pool = ctx.enter_context(tc.tile_pool(name="rb", bufs=2))
    dram_pool = ctx.enter_context(tc.tile_pool(name="scr", bufs=2, space="DRAM"))
    sc_pool = ctx.enter_context(tc.tile_pool(name="sc", bufs=2, space="PSUM"))
    o_pool = ctx.enter_context(tc.tile_pool(name="o", bufs=2, space="PSUM"))
    den_pool = ctx.enter_context(tc.tile_pool(name="den", bufs=1, space="PSUM"))
    acc = ctx.enter_context(tc.tile_pool(name="acc", bufs=1, space="PSUM"))

    wq = persist.tile([128, CC, DQ], BF16, tag="wq")
    wk = persist.tile([128, CC, DQ], BF16, tag="wk")
    wv = persist.tile([128, CC, DQ], BF16, tag="wv")
    wp = persist.tile([128, HC, C], BF16, tag="wp")
    ones32 = persist.tile([128, 32], BF16, tag="ones32")
    nc.vector.memset(ones32[:], 1.0)

    qTs = [persist.tile([128, N], BF16, tag=f"qT{i}", name=f"qT{i}")
           for i in range(HC)]
    kTs = [persist.tile([128, N], BF16, tag=f"kT{i}", name=f"kT{i}")
           for i in range(HC)]
    vs = [persist.tile([128, DQ], BF16, tag=f"v{i}", name=f"v{i}")
          for i in range(KC)]
    hT = persist.tile([128, HC, N], BF16, tag="hT")

    xt_pool = ctx.enter_context(tc.tile_pool(name="xt", bufs=1))
    xts = [xt_pool.tile([128, N], BF16, tag=f"xt{i}", name=f"xt{i}")
           for i in range(CC)]

    # DMA order: first what gates the first exp (wq, wk, x tb0), then wv
    # (V fillers start in the first q-block), then the rest of x, then wp
    xT_r = xT.ap().rearrange("(cc p) t -> p cc t", p=128)
    wq_r = wqT.ap().rearrange("(cc p) d -> p cc d", p=128)
    wk_r = wkT.ap().rearrange("(cc p) d -> p cc d", p=128)
    wv_r = wvT.ap().rearrange("(cc p) d -> p cc d", p=128)
    for cc in range(CC):
        nc.sync.dma_start(out=wq[:, cc, :], in_=wq_r[:, cc, :])
        nc.sync.dma_start(out=wk[:, cc, :], in_=wk_r[:, cc, :])
        nc.sync.dma_start(out=xts[cc][:, 0:512], in_=xT_r[:, cc, 0:512])
    for cc in range(CC):
        nc.sync.dma_start(out=wv[:, cc, :], in_=wv_r[:, cc, :])
    for tb in range(1, QB):
        for cc in range(CC):
            nc.sync.dma_start(
                out=xts[cc][:, tb * 512:(tb + 1) * 512],
                in_=xT_r[:, cc, tb * 512:(tb + 1) * 512])
    nc.sync.dma_start(
        out=wp[:], in_=wpT.ap().rearrange("(hc p) o -> p hc o", p=128))

    # ---- filler piece machinery ------------------------------------------
    # every piece is <= ~2 matmuls (~430ns of PE) so units never overload

    def qk_pieces(hc, tbs=None, which=("q", "k")):
        """K/Q tile construction, 2 accumulating matmuls per piece."""
        for tb in (range(QB) if tbs is None else tbs):
            for w in which:
                w_sb, dst = (wq, qTs[hc]) if w == "q" else (wk, kTs[hc])
                state = {}

                def mk(cc0, w_sb=w_sb, dst=dst, tb=tb, state=state):
                    def piece():
                        if cc0 == 0:
                            state["ps"] = acc.tile(
                                [128, 512], F32, tag="acc", name="qkps")
                        ps = state["ps"]
                        for cc in (cc0, cc0 + 1):
                            nc.tensor.matmul(
                                ps[:],
                                w_sb[:, cc, hc * 128:(hc + 1) * 128],
                                xts[cc][:, tb * 512:(tb + 1) * 512],
                                start=(cc == 0), stop=(cc == CC - 1))
                        if cc0 == CC - 2:
                            nc.vector.tensor_copy(
                                dst[:, tb * 512:(tb + 1) * 512], ps[:])
                    return piece
                for cc0 in range(0, CC, 2):
                    yield mk(cc0)

    def v_pieces(tci, half):
        """One half (256 v-dims) of V chunk tci, 2 matmuls per piece."""
        cs = slice(half * 256, half * 256 + 256)
        state = {}

        def mk(cc0):
            def piece():
                if cc0 == 0:
                    state["ps"] = acc.tile(
                        [128, 512], F32, tag="acc", name="vps")
                ps = state["ps"]
                for cc in (cc0, cc0 + 1):
                    nc.tensor.matmul(
                        ps[0:128, 0:256],
                        xts[cc][:, tci * 128:(tci + 1) * 128],
                        wv[:, cc, cs],
                        start=(cc == 0), stop=(cc == CC - 1))
                if cc0 == CC - 2:
                    nc.vector.tensor_copy(vs[tci][:, cs], ps[0:128, 0:256])
            return piece
        for cc0 in range(0, CC, 2):
            yield mk(cc0)

    def proj_pieces(hc, tb):
        # partial projection for head pair hc over token block tb:
        # OUT^T_hc[ot-block] = Wp[hc] @ H^T[hc]; host sums the partials
        outP_r = outP.ap()[hc].rearrange("(ot p) t -> p ot t", p=128)
        for ot in range(OT):
            def piece(ot=ot):
                ps = acc.tile([128, 512], F32, tag="acc", name="prps")
                nc.tensor.matmul(
                    ps[:],
                    wp[:, hc, ot * 128:(ot + 1) * 128],
                    hT[:, hc, tb * 512:(tb + 1) * 512],
                    start=True, stop=True)
                so = stg_pool.tile([128, 512], F32, tag="stg", name="so")
                nc.vector.tensor_copy(so[:], ps[:])
                nc.sync.dma_start(
                    out=outP_r[:, ot, tb * 512:(tb + 1) * 512], in_=so[:])
            yield piece

    # ---- attention stream -------------------------------------------------

    def den_slot(den_ps, p_a, p_b, start, stop):
        # softmax1 denominator partials: 4 column-tiled M=32 matmuls, one
        # per (k-chunk, head), all ones-stationary, running concurrently
        # in disjoint PE column groups. Row layout of den_ps:
        #   0-31: head0 (even kc)  32-63: head0 (odd kc)
        #  64-95: head1 (even kc)  96-127: head1 (odd kc)
        for hp in range(2):
            for j, p_x in enumerate((p_a, p_b)):
                cp = 64 * hp + 32 * j
                nc.tensor.matmul(
                    den_ps[cp:cp + 32, :], ones32[:], p_x[:, hp, :],
                    start=start, stop=stop, tile_position=(0, cp))

    def emit_attention(hc, unit_fillers):
        """unit_fillers: list of per-unit lists of filler pieces, indexed
        by qb*KC + kc; extra pieces run after the unit's own work."""
        for qb in range(QB):
            qs = slice(qb * 512, (qb + 1) * 512)
            o_ps = o_pool.tile([128, 512], F32, tag="o", name=f"o{hc}_{qb}")
            den_ps = den_pool.tile([128, 512], F32, tag="den", name="den")
            ps = []  # live p tiles, ps[kc]
            for kc in range(KC):
                s_ps = sc_pool.tile([128, 2, 512], F32, tag="sc")
                for hp in range(2):
                    ho = hp * 64
                    nc.tensor.matmul(
                        s_ps[:, hp, :],
                        kTs[hc][ho:ho + 64, kc * 128:(kc + 1) * 128],
                        qTs[hc][ho:ho + 64, qs],
                        start=True, stop=True)
                p_sb = p_pool.tile([128, 2, 512], BF16, tag="p")
                nc.scalar.activation(
                    out=p_sb[:], in_=s_ps[:], func=AF.Exp, scale=SCALE)
                ps.append(p_sb)
                if kc >= 1:
                    pk = kc - 1
                    for hp in range(2):
                        nc.tensor.matmul(
                            o_ps[64 * hp:64 * hp + 64, :],
                            vs[pk][:, 128 * hc + 64 * hp:
                                   128 * hc + 64 * hp + 64],
                            ps[pk][:, hp, :],
                            start=(pk == 0), stop=False,
                            tile_position=(0, 64 * hp))
                if kc >= 2 and kc % 2 == 0:
                    den_slot(den_ps, ps[kc - 2], ps[kc - 1],
                             start=(kc == 2), stop=False)
                for piece in unit_fillers[qb * KC + kc]:
                    piece()
            # drain: last attnV chunk + final denominator slot
            for hp in range(2):
                nc.tensor.matmul(
                    o_ps[64 * hp:64 * hp + 64, :],
                    vs[KC - 1][:, 128 * hc + 64 * hp:
                               128 * hc + 64 * hp + 64],
                    ps[KC - 1][:, hp, :],
                    start=False, stop=True,
                    tile_position=(0, 64 * hp))
            den_slot(den_ps, ps[KC - 2], ps[KC - 1], start=False, stop=True)

            # normalization: recip = 1/(1 + den), broadcast over the 64
            # head-dim partitions, multiply O^T out of PSUM into H^T
            # den_ps -> SBUF (DVE reads only one PSUM operand), then DMA
            # partition-gathers so the even/odd partials line up on the
            # same lanes (DVE cannot shift partitions), then combine
            u_t = u_pool.tile([128, 512], F32, tag="du", name="du")
            nc.vector.tensor_copy(u_t[:], den_ps[:])
            a_t = s_pool.tile([64, 2, 512], F32, tag="dsb", name="dsb")
            nc.gpsimd.dma_start(out=a_t[0:32, 0, :], in_=u_t[0:32, :])
            nc.gpsimd.dma_start(out=a_t[32:64, 0, :], in_=u_t[64:96, :])
            nc.gpsimd.dma_start(out=a_t[0:32, 1, :], in_=u_t[32:64, :])
            nc.gpsimd.dma_start(out=a_t[32:64, 1, :], in_=u_t[96:128, :])
            nc.vector.tensor_add(a_t[:, 0, :], a_t[:, 0, :], a_t[:, 1, :])
            nc.vector.tensor_scalar_add(a_t[:, 0, :], a_t[:, 0, :], 1.0)
            nc.vector.reciprocal(a_t[:, 0, :], a_t[:, 0, :])
            # partition-broadcast bounce: DVE lanes can't shift partitions
            # and SBUF sources can't have stride-0 partition APs, so the
            # recips go out to DRAM and come back broadcast 64-wide
            r_dram = dram_pool.tile([2, 512], F32, tag="rdram", name="rdram")
            for hp in range(2):
                nc.gpsimd.dma_start(
                    out=r_dram[hp:hp + 1, :],
                    in_=a_t[32 * hp:32 * hp + 1, 0, :])
            rb_t = rb_pool.tile([128, 512], F32, tag="rb", name="rb")
            for hp in range(2):
                nc.gpsimd.dma_start(
                    out=rb_t[64 * hp:64 * hp + 64, :],
                    in_=r_dram[hp:hp + 1, :].broadcast_to((64, 512)))
            nc.vector.tensor_mul(hT[:, hc, qs], o_ps[:], rb_t[:])

    # ---- static filler schedule ------------------------------------------
    def spread(units, pieces):
        """Distribute pieces round-robin over the given unit slots."""
        pieces = list(pieces)
        for i, piece in enumerate(pieces):
            units[i * len(units) // len(pieces)].append(piece)

    def unit_lists():
        return [[] for _ in range(QB * KC)]

    # pair 0: qb0 carries the first-half V chunks (needed by its own
    # attnV); qb1-3 carry the K/Q build of pair 1.
    # v half-A for chunk kc must be done before unit kc+1; pack its 4
    # pieces into units kc-1 and kc (2 each), clamped at the start
    uf0 = unit_lists()
    for kc in range(KC):
        pieces = list(v_pieces(kc, 0))
        slots = [max(kc - 1, 0), kc]
        for i, piece in enumerate(pieces):
            uf0[slots[i * len(slots) // len(pieces)]].append(piece)
    spread([uf0[u] for u in range(KC, QB * KC)], qk_pieces(1))

    # pair 1: K/Q of pair 2 + second-half V chunks
    uf1 = unit_lists()
    spread([uf1[u] for u in range(0, QB * KC, 2)], qk_pieces(2))
    vb = []
    for kc in range(KC):
        vb.extend(v_pieces(kc, 1))
    spread([uf1[u] for u in range(1, QB * KC, 2)], vb)

    # pair 2: K/Q of pair 3 + projection of pair 0 (hT[0] ready per qb as
    # pair 0's norms completed long ago)
    uf2 = unit_lists()
    spread([uf2[u] for u in range(0, QB * KC, 2)], qk_pieces(3))
    pr0 = []
    for tb in range(QB):
        pr0.extend(proj_pieces(0, tb))
    spread([uf2[u] for u in range(1, QB * KC, 2)], pr0)

    # pair 3: projection of pairs 1 and 2; its own projection trails one
    # q-block behind (hT[3][tb] ready only after norm(3, tb))
    uf3 = unit_lists()
    pr12 = []
    for tb in range(QB):
        pr12.extend(proj_pieces(1, tb))
        pr12.extend(proj_pieces(2, tb))
    spread([uf3[u] for u in range(0, QB * KC)], pr12)
    for qb in range(1, QB):
        spread([uf3[u] for u in range(qb * KC, (qb + 1) * KC)],
               proj_pieces(3, qb - 1))

    # ---- emission ---------------------------------------------------------
    # K first (its first chunks gate the first scores), then Q, pair 0
    for piece in qk_pieces(0, tbs=(0,), which=("k",)):
        piece()
    for piece in qk_pieces(0, tbs=(0,), which=("q",)):
        piece()
    for piece in qk_pieces(0, tbs=(1, 2, 3)):
        piece()

    emit_attention(0, uf0)
    emit_attention(1, uf1)
    emit_attention(2, uf2)
    emit_attention(3, uf3)
    for piece in proj_pieces(3, QB - 1):
        piece()


_CACHED = None


def _get_nc():
    global _CACHED
    if _CACHED is None:
        nc = bass.Bass("TRN2", target_bir_lowering=False, debug=False)
        xT = nc.dram_tensor("xT", [C, N], BF16, kind="ExternalInput")
        wqT = nc.dram_tensor("wqT", [C, DQ], BF16, kind="ExternalInput")
        wkT = nc.dram_tensor("wkT", [C, DQ], BF16, kind="ExternalInput")
        wvT = nc.dram_tensor("wvT", [C, DQ], BF16, kind="ExternalInput")
        wpT = nc.dram_tensor("wpT", [DQ, C], BF16, kind="ExternalInput")
        outP = nc.dram_tensor("outP", [HC, C, N], F32, kind="ExternalOutput")
        with tile.TileContext(nc) as tc:
            with ExitStack() as ctx:
                _build(ctx, tc, xT, wqT, wkT, wvT, wpT, outP)
        _split_excess_waits(nc)
        _CACHED = nc
    return _CACHED


def run(x, mask, qkv_w, proj_w, proj_b, trace=False):
    x = np.asarray(x, dtype=np.float32)
    qkv_w = np.asarray(qkv_w, dtype=np.float32)
    proj_w = np.asarray(proj_w, dtype=np.float32)
    proj_b = np.asarray(proj_b, dtype=np.float32)

    in_maps = []
    for core in range(N_CORES):
        b, g = core // 2, core % 2
        r = slice(512 * g, 512 * g + 512)
        in_maps.append({
            "xT": np.ascontiguousarray(x[b].T).astype(NPBF16),
            "wqT": np.ascontiguousarray(qkv_w[r].T).astype(NPBF16),
            "wkT": np.ascontiguousarray(qkv_w[1024:][r].T).astype(NPBF16),
            "wvT": np.ascontiguousarray(qkv_w[2048:][r].T).astype(NPBF16),
            "wpT": np.ascontiguousarray(proj_w[:, r].T).astype(NPBF16),
        })

    nc = _get_nc()
    res = run_bass_kernel_spmd(
        nc, in_maps, core_ids=list(range(N_CORES)), trace=trace)

    out = np.empty((B, N, C), dtype=np.float32)
    for b in range(B):
        acc_np = (res.results[2 * b]["outP"].sum(axis=0)
                  + res.results[2 * b + 1]["outP"].sum(axis=0))
        out[b] = acc_np.T + proj_b
    return out, res


def kernel(x, mask, qkv_w, proj_w, proj_b):
    out, _ = run(x, mask, qkv_w, proj_w, proj_b, trace=False)
    return out


# revision 13
# speedup vs baseline: 1.0988x; 1.0988x over previous
"""Trainium2 Bass kernel for nn_Attention (B=4, N=2048, dim=1024, 16 heads).

Sharding: each of the 8 cores handles one (batch, head-group) pair —
batch b = core//2, head-group g = core%2 (8 heads each). Per core:
  qkv part  : Q^T,K^T = W_{q,k}[g] @ x_b^T (d-major), V natural
              (+ a ones column per head for the softmax1 denominator)
  attention : S^T = K^T-chunk.T @ Q^T-padded (full 128-row contraction;
              each head's Q^T lives in its own 128-row tile with the
              other head's rows zeroed, so every matmul in the kernel
              shares one PE config and stationary reloads stay hidden
              in the background weight buffer),
              P^T = exp(S^T * scale)  (no max subtraction: logits ~ N(0,1)),
              O^T[d,q] (+denom row) = [V|1].T @ P^T  accumulated over k,
              software-pipelined one k-chunk behind the exp stream
  normalize : recip = exp-free 1/(1+denom) on DVE, partition-broadcast
              via DRAM bounce, multiply
  proj      : OUT^T = Wp[g]-slices @ H^T   (partial over head channels)
Host side: per batch, out[b] = (OUT^T_{2b} + OUT^T_{2b+1}).T + proj_b.

All non-score PE work (QKV build, V build, projection) is emitted as
~2-matmul filler pieces spread evenly across the exp-paced units.
"""

import numpy as np
import ml_dtypes
from contextlib import ExitStack

import concourse.bass as bass
import concourse.tile as tile
from concourse import mybir
from concourse.bass_utils import run_bass_kernel_spmd

BF16 = mybir.dt.bfloat16
F32 = mybir.dt.float32
AF = mybir.ActivationFunctionType
NPBF16 = ml_dtypes.bfloat16

N_CORES = 8
B = 4
N = 2048          # tokens per batch
C = 1024          # model dim
NH = 8            # heads per core
HD = 64           # head dim
DQ = NH * HD      # q/k/v dims per core (512)
SCALE = HD ** -0.5
CC = C // 128     # contraction chunks (8)
QB = N // 512     # q blocks of 512 (4)
KC = N // 128     # k chunks of 128 (16)
HC = DQ // 128    # head pairs (4)
OT = C // 128     # output row tiles (8)
_MAX_WAITS = 1


def _split_excess_waits(nc):
    """This walrus build rejects >1 semaphore wait per instruction
    ("Too many sync wait commands"); move the excess onto NOPs inserted
    immediately before the offending instruction on the same engine."""
    n_new = 0
    for f in nc.m.functions:
        for bb in f.blocks:
            insts = bb.instructions
            i = 0
            while i < len(insts):
                inst = insts[i]
                si = inst.sync_info
                if si is not None and si.on_wait and len(si.on_wait) > _MAX_WAITS:
                    waits = list(si.on_wait)
                    keep, rest = waits[:_MAX_WAITS], waits[_MAX_WAITS:]
                    nops = []
                    while rest:
                        chunk, rest = rest[:_MAX_WAITS], rest[_MAX_WAITS:]
                        nop = mybir.InstNoOp(
                            name=f"wait-split-{n_new}", ins=[], outs=[])
                        n_new += 1
                        nop.engine = inst.engine
                        nop.sync_info = mybir.SyncInfo(on_wait=chunk, on_update=[])
                        nops.append(nop)
                    inst.sync_info = mybir.SyncInfo(
                        on_wait=keep, on_update=list(si.on_update or []))
                    for j, nop in enumerate(nops):
                        insts.insert(i + j, nop)
                    i += len(nops)
                i += 1
    return n_new


def _build(ctx: ExitStack, tc: tile.TileContext, xT, wqT, wkT, wvT, wpT, outP):
    nc = tc.nc

    persist = ctx.enter_context(tc.tile_pool(name="persist", bufs=1))
    p_pool = ctx.enter_context(tc.tile_pool(name="p", bufs=8))
    stg_pool = ctx.enter_context(tc.tile_pool(name="stg", bufs=3))
    den_pool = ctx.enter_context(tc.tile_pool(name="den2", bufs=2))
    dram_pool = ctx.enter_context(tc.tile_pool(name="scr", bufs=1, space="DRAM"))
    acc = ctx.enter_context(tc.tile_pool(name="acc", bufs=2, space="PSUM"))
    opair = ctx.enter_context(tc.tile_pool(name="opair", bufs=2, space="PSUM"))
    sc_pool = ctx.enter_context(tc.tile_pool(name="sc", bufs=2, space="PSUM"))

    wq = persist.tile([128, CC, DQ], BF16, tag="wq")
    wk = persist.tile([128, CC, DQ], BF16, tag="wk")
    wv = persist.tile([128, CC, DQ], BF16, tag="wv")
    wp = persist.tile([128, HC, C], BF16, tag="wp")

    # per-head-padded Q^T tiles: head hp of pair hc occupies rows
    # 64*hp..64*hp+63; the other 64 rows are zero so a full-128-row
    # matmul against the pair's K^T chunk yields that head's scores
    qTs = [[persist.tile([128, N], BF16, tag=f"qT{i}_{hp}",
                         name=f"qT{i}_{hp}") for hp in range(2)]
           for i in range(HC)]
    kTs = [persist.tile([128, N], BF16, tag=f"kT{i}", name=f"kT{i}")
           for i in range(HC)]
    vs = [persist.tile([128, NH * (HD + 1)], BF16, tag=f"v{i}", name=f"v{i}")
          for i in range(KC)]
    hT = persist.tile([128, HC, N], BF16, tag="hT")

    den_drams = [dram_pool.tile([2, N], F32, tag=f"dend{i}", name=f"dend{i}")
                 for i in range(HC)]
    recip_drams = [dram_pool.tile([2, N], F32, tag=f"recd{i}", name=f"recd{i}")
                   for i in range(HC)]

    # ones columns for the softmax1 denominator (written once; V copies
    # below only overwrite the 64-wide per-head value slices)
    for v_t in vs:
        nc.vector.memset(v_t[:], 1.0)
    # zero halves of the padded Q^T tiles (written once)
    for hc in range(HC):
        nc.vector.memset(qTs[hc][0][64:128, :], 0.0)
        nc.vector.memset(qTs[hc][1][0:64, :], 0.0)

    oT_pool = ctx.enter_context(tc.tile_pool(name="oT", bufs=2))
    xt_pool = ctx.enter_context(tc.tile_pool(name="xt", bufs=1))
    rb_pool = ctx.enter_context(tc.tile_pool(name="rb", bufs=2))
    xts = [xt_pool.tile([128, N], BF16, tag=f"xt{i}", name=f"xt{i}")
           for i in range(CC)]

    # DMA order: wq, wk and x tb0 gate the first scores; wv next (V
    # fillers run during the first q-block); then the rest of x, then wp
    xT_r = xT.ap().rearrange("(cc p) t -> p cc t", p=128)
    wq_r = wqT.ap().rearrange("(cc p) d -> p cc d", p=128)
    wk_r = wkT.ap().rearrange("(cc p) d -> p cc d", p=128)
    wv_r = wvT.ap().rearrange("(cc p) d -> p cc d", p=128)
    for cc in range(CC):
        nc.sync.dma_start(out=wq[:, cc, :], in_=wq_r[:, cc, :])
        nc.sync.dma_start(out=wk[:, cc, :], in_=wk_r[:, cc, :])
        nc.sync.dma_start(out=xts[cc][:, 0:512], in_=xT_r[:, cc, 0:512])
    for cc in range(CC):
        nc.sync.dma_start(out=wv[:, cc, :], in_=wv_r[:, cc, :])
    for tb in range(1, QB):
        for cc in range(CC):
            nc.sync.dma_start(
                out=xts[cc][:, tb * 512:(tb + 1) * 512],
                in_=xT_r[:, cc, tb * 512:(tb + 1) * 512])
    nc.sync.dma_start(
        out=wp[:], in_=wpT.ap().rearrange("(hc p) o -> p hc o", p=128))

    # ---- filler piece machinery (all matmuls share the 128x128 config,
    # each piece <= ~2 matmuls so no unit overloads the PE) -------------

    def qk_pieces(hc, tbs=None, which=("q", "k")):
        for tb in (range(QB) if tbs is None else tbs):
            for w in which:
                w_sb = wq if w == "q" else wk
                state = {}

                def mk(cc0, w=w, w_sb=w_sb, tb=tb, state=state, hc=hc):
                    def piece():
                        if cc0 == 0:
                            state["ps"] = acc.tile(
                                [128, 512], F32, tag="acc", name="qkps")
                        ps = state["ps"]
                        for cc in (cc0, cc0 + 1):
                            nc.tensor.matmul(
                                ps[:],
                                w_sb[:, cc, hc * 128:(hc + 1) * 128],
                                xts[cc][:, tb * 512:(tb + 1) * 512],
                                start=(cc == 0), stop=(cc == CC - 1))
                        if cc0 == CC - 2:
                            ts = slice(tb * 512, (tb + 1) * 512)
                            if w == "q":
                                nc.vector.tensor_copy(
                                    qTs[hc][0][0:64, ts], ps[0:64, :])
                                nc.vector.tensor_copy(
                                    qTs[hc][1][64:128, ts], ps[64:128, :])
                            else:
                                nc.vector.tensor_copy(kTs[hc][:, ts], ps[:])
                    return piece
                for cc0 in range(0, CC, 2):
                    yield mk(cc0)

    def v_pieces(tci):
        state = {}

        def mk(cc0):
            def piece():
                if cc0 == 0:
                    state["ps"] = acc.tile(
                        [128, 512], F32, tag="acc", name="vps")
                ps = state["ps"]
                for cc in (cc0, cc0 + 1):
                    nc.tensor.matmul(
                        ps[:],
                        xts[cc][:, tci * 128:(tci + 1) * 128],
                        wv[:, cc, :],
                        start=(cc == 0), stop=(cc == CC - 1))
                if cc0 == CC - 2:
                    nc.vector.tensor_copy(
                        vs[tci][:].rearrange(
                            "p (h e) -> p h e", e=HD + 1)[:, :, 0:HD],
                        ps[:].rearrange("p (h e) -> p h e", e=HD))
            return piece
        for cc0 in range(0, CC, 2):
            yield mk(cc0)

    def proj_pieces(hc, tb):
        outP_r = outP.ap()[hc].rearrange("(ot p) t -> p ot t", p=128)
        for ot in range(OT):
            def piece(ot=ot):
                ps = acc.tile([128, 512], F32, tag="acc", name="prps")
                nc.tensor.matmul(
                    ps[:],
                    wp[:, hc, ot * 128:(ot + 1) * 128],
                    hT[:, hc, tb * 512:(tb + 1) * 512],
                    start=True, stop=True)
                so = stg_pool.tile([128, 512], F32, tag="stg", name="so")
                nc.vector.tensor_copy(so[:], ps[:])
                nc.sync.dma_start(
                    out=outP_r[:, ot, tb * 512:(tb + 1) * 512], in_=so[:])
            yield piece

    oT_tiles = {}

    def norm_qb(hc, qb):
        # recip = 1/(1+den) for one q-block of pair hc (DVE + DMA only;
        # no PE work). den rows were extracted to DRAM by emit_attention.
        qs = slice(qb * 512, (qb + 1) * 512)
        den2 = den_pool.tile([128, 8], F32, tag="den2", name="den2")
        nc.gpsimd.dma_start(
            out=den2[:],
            in_=den_drams[hc][:, qs].rearrange("h (a i) -> h a i", i=8))
        nc.vector.tensor_scalar_add(den2[:], den2[:], 1.0)
        nc.vector.reciprocal(den2[:], den2[:])
        nc.gpsimd.dma_start(
            out=recip_drams[hc][:, qs].rearrange("h (a i) -> h a i", i=8),
            in_=den2[:])
        rb_t = rb_pool.tile([128, 512], F32, tag="rb", name="rb")
        for half in range(2):
            src = recip_drams[hc][half:half + 1, qs].broadcast_to((64, 512))
            nc.gpsimd.dma_start(out=rb_t[half * 64:(half + 1) * 64, :], in_=src)
        nc.vector.tensor_mul(
            hT[:, hc, qs], oT_tiles[hc][:, qs], rb_t[:])

    def emit_attention(hc, unit_fillers, unit_hooks=None):
        """unit_fillers[qb*KC+kc]: filler pieces to run in that unit.
        unit_hooks: optional dict {unit_index: callable} for norm/proj
        staggering of the final pair."""
        oT_t = oT_pool.tile([128, N], F32, tag="oT", name=f"oT{hc}")
        oT_tiles[hc] = oT_t
        vcols = [(2 * hc + hp) * (HD + 1) for hp in range(2)]
        for qb in range(QB):
            qs = slice(qb * 512, (qb + 1) * 512)
            o_ps = [opair.tile([128, 512], F32, tag="opair", name=f"ops{hp}")
                    for hp in range(2)]

            def attn_chunk(kc, p_sb):
                for hp in range(2):
                    nc.tensor.matmul(
                        o_ps[hp][0:HD + 1, :],
                        vs[kc][:, vcols[hp]:vcols[hp] + HD + 1],
                        p_sb[:, hp, :],
                        start=(kc == 0), stop=(kc == KC - 1))

            prev = None
            for kc in range(KC):
                u = qb * KC + kc
                if unit_hooks and u in unit_hooks:
                    unit_hooks[u]()
                # scores: full-128-row matmuls against the padded Q^T
                # tiles — same PE config as every other matmul here
                s_ps = sc_pool.tile([128, 2, 512], F32, tag="sc")
                for hp in range(2):
                    nc.tensor.matmul(
                        s_ps[:, hp, :],
                        kTs[hc][:, kc * 128:(kc + 1) * 128],
                        qTs[hc][hp][:, qs],
                        start=True, stop=True)
                p_sb = p_pool.tile([128, 2, 512], BF16, tag="p")
                nc.scalar.activation(
                    out=p_sb[:], in_=s_ps[:], func=AF.Exp, scale=SCALE)
                if prev is not None:
                    attn_chunk(*prev)
                for piece in unit_fillers[u]:
                    piece()
                prev = (kc, p_sb)
            attn_chunk(*prev)

            # drain O^T + denominator rows (baseline mechanics: head 0
            # lands in place; head 1 needs a partition shift via DMA)
            for hp in range(2):
                stg = stg_pool.tile([128, 512], F32, tag="stg")
                if hp == 0:
                    nc.vector.tensor_copy(oT_t[0:HD, qs], o_ps[0][0:HD, :])
                    nc.vector.tensor_copy(
                        stg[HD:HD + 1, :], o_ps[0][HD:HD + 1, :])
                else:
                    nc.vector.tensor_copy(
                        stg[0:HD + 1, :], o_ps[1][0:HD + 1, :])
                    nc.sync.dma_start(
                        out=oT_t[HD:2 * HD, qs], in_=stg[0:HD, :])
                nc.gpsimd.dma_start(
                    out=den_drams[hc][hp:hp + 1, qs], in_=stg[HD:HD + 1, :])

    # ---- static filler schedule ------------------------------------------
    def spread(units, pieces):
        pieces = list(pieces)
        if not pieces:
            return
        for i, piece in enumerate(pieces):
            units[i * len(units) // len(pieces)].append(piece)

    def unit_lists():
        return [[] for _ in range(QB * KC)]

    # pair 0: qb0 must build V (one chunk per unit, just ahead of its
    # first consumer); qb1-3 carry the K/Q build of pair 1
    uf0 = unit_lists()
    for kc in range(KC):
        pieces = list(v_pieces(kc))
        slots = [max(kc - 1, 0), kc]
        for i, piece in enumerate(pieces):
            uf0[slots[i * len(slots) // len(pieces)]].append(piece)
    spread([uf0[u] for u in range(KC, QB * KC)], qk_pieces(1))

    # pair 1: K/Q of pair 2; pair 2: K/Q of pair 3 + projection of
    # pairs 0; pair 3: projection of pairs 1, 2 (+ its own, staggered)
    uf1 = unit_lists()
    spread([uf1[u] for u in range(QB * KC)], qk_pieces(2))
    uf2 = unit_lists()
    pr0 = []
    for tb in range(QB):
        pr0.extend(proj_pieces(0, tb))
    spread([uf2[u] for u in range(0, QB * KC, 2)], qk_pieces(3))
    spread([uf2[u] for u in range(1, QB * KC, 2)], pr0)
    uf3 = unit_lists()
    pr12 = []
    for tb in range(QB):
        pr12.extend(proj_pieces(1, tb))
        pr12.extend(proj_pieces(2, tb))
    spread([uf3[u] for u in range(QB * KC)], pr12)
    # pair 3's own normalization + projection, one q-block behind,
    # the 8 projection pieces spread one per unit
    hooks3 = {}
    for qb in range(1, QB):
        hooks3[qb * KC + 2] = (lambda qb=qb: norm_qb(3, qb - 1))
        for i, piece in enumerate(proj_pieces(3, qb - 1)):
            hooks3[qb * KC + 4 + i] = piece

    # norms for pairs 0-2 run at the start of the NEXT pair's stream
    # (hT[hc] must be ready before proj(hc) fillers); they cost no PE
    def norm_hooks(hc):
        hooks = {}
        for qb in range(QB):
            def hook(hc=hc, qb=qb):
                norm_qb(hc, qb)
            hooks[qb * 4 + 2] = hook  # early units of the next pass
        return hooks

    # ---- emission ---------------------------------------------------------
    for piece in qk_pieces(0, tbs=(0,), which=("k",)):
        piece()
    for piece in qk_pieces(0, tbs=(0,), which=("q",)):
        piece()
    for piece in qk_pieces(0, tbs=(1, 2, 3)):
        piece()

    emit_attention(0, uf0)
    emit_attention(1, uf1, unit_hooks=norm_hooks(0))
    emit_attention(2, uf2, unit_hooks=norm_hooks(1))
    emit_attention(3, uf3, unit_hooks={**norm_hooks(2), **hooks3})
    norm_qb(3, QB - 1)
    for piece in proj_pieces(3, QB - 1):
        piece()


_CACHED = None


def _get_nc():
    global _CACHED
    if _CACHED is None:
        nc = bass.Bass("TRN2", target_bir_lowering=False, debug=False)
        xT = nc.dram_tensor("xT", [C, N], BF16, kind="ExternalInput")
        wqT = nc.dram_tensor("wqT", [C, DQ], BF16, kind="ExternalInput")
        wkT = nc.dram_tensor("wkT", [C, DQ], BF16, kind="ExternalInput")
        wvT = nc.dram_tensor("wvT", [C, DQ], BF16, kind="ExternalInput")
        wpT = nc.dram_tensor("wpT", [DQ, C], BF16, kind="ExternalInput")
        outP = nc.dram_tensor("outP", [HC, C, N], F32, kind="ExternalOutput")
        with tile.TileContext(nc) as tc:
            with ExitStack() as ctx:
                _build(ctx, tc, xT, wqT, wkT, wvT, wpT, outP)
        _split_excess_waits(nc)
        _CACHED = nc
    return _CACHED


def run(x, mask, qkv_w, proj_w, proj_b, trace=False):
    x = np.asarray(x, dtype=np.float32)
    qkv_w = np.asarray(qkv_w, dtype=np.float32)
    proj_w = np.asarray(proj_w, dtype=np.float32)
    proj_b = np.asarray(proj_b, dtype=np.float32)

    in_maps = []
    for core in range(N_CORES):
        b, g = core // 2, core % 2
        r = slice(512 * g, 512 * g + 512)
        in_maps.append({
            "xT": np.ascontiguousarray(x[b].T).astype(NPBF16),
            "wqT": np.ascontiguousarray(qkv_w[r].T).astype(NPBF16),
            "wkT": np.ascontiguousarray(qkv_w[1024:][r].T).astype(NPBF16),
            "wvT": np.ascontiguousarray(qkv_w[2048:][r].T).astype(NPBF16),
            "wpT": np.ascontiguousarray(proj_w[:, r].T).astype(NPBF16),
        })

    nc = _get_nc()
    res = run_bass_kernel_spmd(
        nc, in_maps, core_ids=list(range(N_CORES)), trace=trace)

    out = np.empty((B, N, C), dtype=np.float32)
    for b in range(B):
        acc_np = (res.results[2 * b]["outP"].sum(axis=0)
                  + res.results[2 * b + 1]["outP"].sum(axis=0))
        out[b] = acc_np.T + proj_b
    return out, res


def kernel(x, mask, qkv_w, proj_w, proj_b):
    out, _ = run(x, mask, qkv_w, proj_w, proj_b, trace=False)
    return out


# revision 18
# speedup vs baseline: 1.1233x; 1.0223x over previous
"""Trainium2 Bass kernel for nn_Attention (B=4, N=2048, dim=1024, 16 heads).

Sharding: each of the 8 cores handles one (batch, head-group) pair —
batch b = core//2, head-group g = core%2 (8 heads each). Per core:
  qkv part  : Q^T,K^T = W_{q,k}[g] @ x_b^T (d-major), V natural
              (+ a ones column per head for the softmax1 denominator)
  attention : S^T = K^T-chunk.T @ Q^T-padded (full 128-row contraction;
              each head's Q^T lives in its own 128-row tile with the
              other head's rows zeroed, so every matmul in the kernel
              shares one PE config and stationary reloads stay hidden
              in the background weight buffer),
              P^T = exp(S^T * scale)  (no max subtraction: logits ~ N(0,1)),
              O^T[d,q] (+denom row) = [V|1].T @ P^T  accumulated over k,
              software-pipelined one k-chunk behind the exp stream
  normalize : recip = exp-free 1/(1+denom) on DVE, partition-broadcast
              via DRAM bounce, multiply
  proj      : OUT^T = Wp[g]-slices @ H^T   (partial over head channels)
Host side: per batch, out[b] = (OUT^T_{2b} + OUT^T_{2b+1}).T + proj_b.

All non-score PE work (QKV build, V build, projection) is emitted as
~2-matmul filler pieces spread evenly across the exp-paced units.
"""

import numpy as np
import ml_dtypes
from contextlib import ExitStack

import concourse.bass as bass
import concourse.tile as tile
from concourse import mybir
from concourse.bass_utils import run_bass_kernel_spmd

BF16 = mybir.dt.bfloat16
F32 = mybir.dt.float32
AF = mybir.ActivationFunctionType
NPBF16 = ml_dtypes.bfloat16

N_CORES = 8
B = 4
N = 2048          # tokens per batch
C = 1024          # model dim
NH = 8            # heads per core
HD = 64           # head dim
DQ = NH * HD      # q/k/v dims per core (512)
SCALE = HD ** -0.5
CC = C // 128     # contraction chunks (8)
QB = N // 512     # q blocks of 512 (4)
KC = N // 128     # k chunks of 128 (16)
HC = DQ // 128    # head pairs (4)
OT = C // 128     # output row tiles (8)
_MAX_WAITS = 1


def _split_excess_waits(nc):
    """This walrus build rejects >1 semaphore wait per instruction
    ("Too many sync wait commands"); move the excess onto NOPs inserted
    immediately before the offending instruction on the same engine."""
    n_new = 0
    for f in nc.m.functions:
        for bb in f.blocks:
            insts = bb.instructions
            i = 0
            while i < len(insts):
                inst = insts[i]
                si = inst.sync_info
                if si is not None and si.on_wait and len(si.on_wait) > _MAX_WAITS:
                    waits = list(si.on_wait)
                    keep, rest = waits[:_MAX_WAITS], waits[_MAX_WAITS:]
                    nops = []
                    while rest:
                        chunk, rest = rest[:_MAX_WAITS], rest[_MAX_WAITS:]
                        nop = mybir.InstNoOp(
                            name=f"wait-split-{n_new}", ins=[], outs=[])
                        n_new += 1
                        nop.engine = inst.engine
                        nop.sync_info = mybir.SyncInfo(on_wait=chunk, on_update=[])
                        nops.append(nop)
                    inst.sync_info = mybir.SyncInfo(
                        on_wait=keep, on_update=list(si.on_update or []))
                    for j, nop in enumerate(nops):
                        insts.insert(i + j, nop)
                    i += len(nops)
                i += 1
    return n_new


def _build(ctx: ExitStack, tc: tile.TileContext, xT, wqT, wkT, wvT, wpT, outP):
    nc = tc.nc

    persist = ctx.enter_context(tc.tile_pool(name="persist", bufs=1))
    p_pool = ctx.enter_context(tc.tile_pool(name="p", bufs=8))
    stg_pool = ctx.enter_context(tc.tile_pool(name="stg", bufs=3))
    den_pool = ctx.enter_context(tc.tile_pool(name="den2", bufs=2))
    dram_pool = ctx.enter_context(tc.tile_pool(name="scr", bufs=1, space="DRAM"))
    acc = ctx.enter_context(tc.tile_pool(name="acc", bufs=2, space="PSUM"))
    opair = ctx.enter_context(tc.tile_pool(name="opair", bufs=2, space="PSUM"))
    sc_pool = ctx.enter_context(tc.tile_pool(name="sc", bufs=2, space="PSUM"))

    wq = persist.tile([128, CC, DQ], BF16, tag="wq")
    wk = persist.tile([128, CC, DQ], BF16, tag="wk")
    wv = persist.tile([128, CC, DQ], BF16, tag="wv")
    wp = persist.tile([128, HC, C], BF16, tag="wp")

    # per-head-padded Q^T tiles: head hp of pair hc occupies rows
    # 64*hp..64*hp+63; the other 64 rows are zero so a full-128-row
    # matmul against the pair's K^T chunk yields that head's scores
    qTs = [[persist.tile([128, N], BF16, tag=f"qT{i}_{hp}",
                         name=f"qT{i}_{hp}") for hp in range(2)]
           for i in range(HC)]
    kTs = [persist.tile([128, N], BF16, tag=f"kT{i}", name=f"kT{i}")
           for i in range(HC)]
    vs = [persist.tile([128, NH * (HD + 1)], BF16, tag=f"v{i}", name=f"v{i}")
          for i in range(KC)]
    hT = persist.tile([128, HC, N], BF16, tag="hT")

    den_drams = [dram_pool.tile([2, N], F32, tag=f"dend{i}", name=f"dend{i}")
                 for i in range(HC)]
    recip_drams = [dram_pool.tile([2, N], F32, tag=f"recd{i}", name=f"recd{i}")
                   for i in range(HC)]

    # ones columns for the softmax1 denominator (written once; V copies
    # below only overwrite the 64-wide per-head value slices)
    for v_t in vs:
        nc.vector.memset(v_t[:], 1.0)
    # zero halves of the padded Q^T tiles (written once)
    for hc in range(HC):
        nc.vector.memset(qTs[hc][0][64:128, :], 0.0)
        nc.vector.memset(qTs[hc][1][0:64, :], 0.0)

    # PE warmup: ~4.5us of dummy matmuls so the HAM clock gate opens
    # (K=8/8, 2.4 GHz) before the real QKV matmuls start; without this
    # the entire head phase runs at 1.2 GHz
    warm = persist.tile([128, 512], BF16, tag="warm")
    nc.vector.memset(warm[:], 0.0)
    for i in range(24):
        wps = acc.tile([128, 512], F32, tag="acc", name="warmps")
        nc.tensor.matmul(wps[:], warm[:, 0:128], warm[:], start=True,
                         stop=True)

    oT_pool = ctx.enter_context(tc.tile_pool(name="oT", bufs=2))
    xt_pool = ctx.enter_context(tc.tile_pool(name="xt", bufs=1))
    rb_pool = ctx.enter_context(tc.tile_pool(name="rb", bufs=2))
    xts = [xt_pool.tile([128, N], BF16, tag=f"xt{i}", name=f"xt{i}")
           for i in range(CC)]

    # DMA order: wq, wk and x tb0 gate the first scores; wv next (V
    # fillers run during the first q-block); then the rest of x, then wp
    xT_r = xT.ap().rearrange("(cc p) t -> p cc t", p=128)
    wq_r = wqT.ap().rearrange("(cc p) d -> p cc d", p=128)
    wk_r = wkT.ap().rearrange("(cc p) d -> p cc d", p=128)
    wv_r = wvT.ap().rearrange("(cc p) d -> p cc d", p=128)
    for cc in range(CC):
        nc.sync.dma_start(out=wq[:, cc, :], in_=wq_r[:, cc, :])
        nc.sync.dma_start(out=wk[:, cc, :], in_=wk_r[:, cc, :])
        nc.sync.dma_start(out=xts[cc][:, 0:512], in_=xT_r[:, cc, 0:512])
    for cc in range(CC):
        nc.sync.dma_start(out=wv[:, cc, :], in_=wv_r[:, cc, :])
    for tb in range(1, QB):
        for cc in range(CC):
            nc.sync.dma_start(
                out=xts[cc][:, tb * 512:(tb + 1) * 512],
                in_=xT_r[:, cc, tb * 512:(tb + 1) * 512])
    nc.sync.dma_start(
        out=wp[:], in_=wpT.ap().rearrange("(hc p) o -> p hc o", p=128))

    # ---- filler piece machinery (all matmuls share the 128x128 config,
    # each piece <= ~2 matmuls so no unit overloads the PE) -------------

    def qk_pieces(hc, tbs=None, which=("q", "k")):
        for tb in (range(QB) if tbs is None else tbs):
            for w in which:
                w_sb = wq if w == "q" else wk
                state = {}

                def mk(cc0, w=w, w_sb=w_sb, tb=tb, state=state, hc=hc):
                    def piece():
                        if cc0 == 0:
                            state["ps"] = acc.tile(
                                [128, 512], F32, tag="acc", name="qkps")
                        ps = state["ps"]
                        for cc in (cc0, cc0 + 1):
                            nc.tensor.matmul(
                                ps[:],
                                w_sb[:, cc, hc * 128:(hc + 1) * 128],
                                xts[cc][:, tb * 512:(tb + 1) * 512],
                                start=(cc == 0), stop=(cc == CC - 1))
                        if cc0 == CC - 2:
                            ts = slice(tb * 512, (tb + 1) * 512)
                            if w == "q":
                                nc.vector.tensor_copy(
                                    qTs[hc][0][0:64, ts], ps[0:64, :])
                                nc.vector.tensor_copy(
                                    qTs[hc][1][64:128, ts], ps[64:128, :])
                            else:
                                nc.vector.tensor_copy(kTs[hc][:, ts], ps[:])
                    return piece
                for cc0 in range(0, CC, 2):
                    yield mk(cc0)

    def v_pieces(tci):
        state = {}

        def mk(cc0):
            def piece():
                if cc0 == 0:
                    state["ps"] = acc.tile(
                        [128, 512], F32, tag="acc", name="vps")
                ps = state["ps"]
                for cc in (cc0, cc0 + 1):
                    nc.tensor.matmul(
                        ps[:],
                        xts[cc][:, tci * 128:(tci + 1) * 128],
                        wv[:, cc, :],
                        start=(cc == 0), stop=(cc == CC - 1))
                if cc0 == CC - 2:
                    nc.vector.tensor_copy(
                        vs[tci][:].rearrange(
                            "p (h e) -> p h e", e=HD + 1)[:, :, 0:HD],
                        ps[:].rearrange("p (h e) -> p h e", e=HD))
            return piece
        for cc0 in range(0, CC, 2):
            yield mk(cc0)

    def proj_pieces(hc, tb):
        outP_r = outP.ap()[hc].rearrange("(ot p) t -> p ot t", p=128)
        for ot in range(OT):
            def piece(ot=ot):
                ps = acc.tile([128, 512], F32, tag="acc", name="prps")
                nc.tensor.matmul(
                    ps[:],
                    wp[:, hc, ot * 128:(ot + 1) * 128],
                    hT[:, hc, tb * 512:(tb + 1) * 512],
                    start=True, stop=True)
                so = stg_pool.tile([128, 512], F32, tag="stg", name="so")
                nc.vector.tensor_copy(so[:], ps[:])
                nc.sync.dma_start(
                    out=outP_r[:, ot, tb * 512:(tb + 1) * 512], in_=so[:])
            yield piece

    oT_tiles = {}

    def norm_qb(hc, qb):
        # recip = 1/(1+den) for one q-block of pair hc (DVE + DMA only;
        # no PE work). den rows were extracted to DRAM by emit_attention.
        qs = slice(qb * 512, (qb + 1) * 512)
        den2 = den_pool.tile([128, 8], F32, tag="den2", name="den2")
        nc.gpsimd.dma_start(
            out=den2[:],
            in_=den_drams[hc][:, qs].rearrange("h (a i) -> h a i", i=8))
        nc.vector.tensor_scalar_add(den2[:], den2[:], 1.0)
        nc.vector.reciprocal(den2[:], den2[:])
        nc.gpsimd.dma_start(
            out=recip_drams[hc][:, qs].rearrange("h (a i) -> h a i", i=8),
            in_=den2[:])
        rb_t = rb_pool.tile([128, 512], F32, tag="rb", name="rb")
        for half in range(2):
            src = recip_drams[hc][half:half + 1, qs].broadcast_to((64, 512))
            nc.gpsimd.dma_start(out=rb_t[half * 64:(half + 1) * 64, :], in_=src)
        nc.vector.tensor_mul(
            hT[:, hc, qs], oT_tiles[hc][:, qs], rb_t[:])

    def emit_attention(hc, unit_fillers, unit_hooks=None):
        """unit_fillers[qb*KC+kc]: filler pieces to run in that unit.
        unit_hooks: optional dict {unit_index: callable} for norm/proj
        staggering of the final pair."""
        oT_t = oT_pool.tile([128, N], F32, tag="oT", name=f"oT{hc}")
        oT_tiles[hc] = oT_t
        vcols = [(2 * hc + hp) * (HD + 1) for hp in range(2)]
        for qb in range(QB):
            qs = slice(qb * 512, (qb + 1) * 512)
            o_ps = [opair.tile([128, 512], F32, tag="opair", name=f"ops{hp}")
                    for hp in range(2)]

            def attn_chunk(kc, p_sb):
                for hp in range(2):
                    nc.tensor.matmul(
                        o_ps[hp][0:HD + 1, :],
                        vs[kc][:, vcols[hp]:vcols[hp] + HD + 1],
                        p_sb[:, hp, :],
                        start=(kc == 0), stop=(kc == KC - 1))

            # software pipeline depth 2: attnV runs two chunks behind the
            # exp stream, so at q-block boundaries the o_ps bank reuse
            # (gated on the previous block's drain copies) never blocks
            # the next scores in the in-order PE queue
            pend = []
            for kc in range(KC):
                u = qb * KC + kc
                if unit_hooks and u in unit_hooks:
                    unit_hooks[u]()
                # scores: full-128-row matmuls against the padded Q^T
                # tiles — same PE config as every other matmul here
                s_ps = sc_pool.tile([128, 2, 512], F32, tag="sc")
                for hp in range(2):
                    nc.tensor.matmul(
                        s_ps[:, hp, :],
                        kTs[hc][:, kc * 128:(kc + 1) * 128],
                        qTs[hc][hp][:, qs],
                        start=True, stop=True)
                p_sb = p_pool.tile([128, 2, 512], BF16, tag="p")
                nc.scalar.activation(
                    out=p_sb[:], in_=s_ps[:], func=AF.Exp, scale=SCALE)
                pend.append((kc, p_sb))
                if len(pend) > 2:
                    attn_chunk(*pend.pop(0))
                for piece in unit_fillers[u]:
                    piece()
            for item in pend:
                attn_chunk(*item)

            # drain O^T + denominator rows (baseline mechanics: head 0
            # lands in place; head 1 needs a partition shift via DMA)
            for hp in range(2):
                stg = stg_pool.tile([128, 512], F32, tag="stg")
                if hp == 0:
                    nc.vector.tensor_copy(oT_t[0:HD, qs], o_ps[0][0:HD, :])
                    nc.vector.tensor_copy(
                        stg[HD:HD + 1, :], o_ps[0][HD:HD + 1, :])
                else:
                    nc.vector.tensor_copy(
                        stg[0:HD + 1, :], o_ps[1][0:HD + 1, :])
                    nc.sync.dma_start(
                        out=oT_t[HD:2 * HD, qs], in_=stg[0:HD, :])
                nc.gpsimd.dma_start(
                    out=den_drams[hc][hp:hp + 1, qs], in_=stg[HD:HD + 1, :])

    # ---- static filler schedule ------------------------------------------
    def spread(units, pieces):
        pieces = list(pieces)
        if not pieces:
            return
        for i, piece in enumerate(pieces):
            units[i * len(units) // len(pieces)].append(piece)

    def unit_lists():
        return [[] for _ in range(QB * KC)]

    # pair 0: qb0 must build V (one chunk per unit, just ahead of its
    # first consumer — attnV runs 2 behind, so chunk kc is due at unit
    # kc+2); each q-block also builds the next q-block's Q tile; qb1-3
    # carry the K/Q build of pair 1
    uf0 = unit_lists()
    for kc in range(KC):
        pieces = list(v_pieces(kc))
        slots = [max(kc - 1, 0), min(kc + 1, KC - 1)]
        for i, piece in enumerate(pieces):
            uf0[slots[i * len(slots) // len(pieces)]].append(piece)
    for tb in range(1, QB):
        spread([uf0[u] for u in range((tb - 1) * KC + 8, tb * KC)],
               qk_pieces(0, tbs=(tb,), which=("q",)))
    spread([uf0[u] for u in range(KC, QB * KC)], qk_pieces(1))

    # pair 1: K/Q of pair 2; pair 2: K/Q of pair 3 + projection of
    # pairs 0; pair 3: projection of pairs 1, 2 (+ its own, staggered)
    uf1 = unit_lists()
    spread([uf1[u] for u in range(QB * KC)], qk_pieces(2))
    uf2 = unit_lists()
    pr0 = []
    for tb in range(QB):
        pr0.extend(proj_pieces(0, tb))
    spread([uf2[u] for u in range(0, QB * KC, 2)], qk_pieces(3))
    spread([uf2[u] for u in range(1, QB * KC, 2)], pr0)
    uf3 = unit_lists()
    pr12 = []
    for tb in range(QB):
        pr12.extend(proj_pieces(1, tb))
        pr12.extend(proj_pieces(2, tb))
    spread([uf3[u] for u in range(QB * KC)], pr12)
    # pair 3's own normalization + projection, one q-block behind; the
    # norm fires early (its 4-hop DMA chain needs ~4us) and the proj
    # pieces follow 7+ units later so they never head-of-line block
    # the scores stream in the in-order PE queue
    hooks3 = {}
    for qb in range(1, QB):
        hooks3[qb * KC + 1] = (lambda qb=qb: norm_qb(3, qb - 1))
        for i, piece in enumerate(proj_pieces(3, qb - 1)):
            hooks3[qb * KC + 8 + i] = piece

    # norms for pairs 0-2 run at the start of the NEXT pair's stream
    # (hT[hc] must be ready before proj(hc) fillers); they cost no PE
    def norm_hooks(hc):
        hooks = {}
        for qb in range(QB):
            def hook(hc=hc, qb=qb):
                norm_qb(hc, qb)
            hooks[qb * 4 + 2] = hook  # early units of the next pass
        return hooks

    # ---- emission ---------------------------------------------------------
    # minimal head: the full K tile of pair 0 (scores chunk kc needs K
    # columns kc*128.. as the stream advances) and q-block 0's Q tile;
    # the other Q blocks are built as fillers one q-block ahead
    for piece in qk_pieces(0, tbs=(0, 1, 2, 3), which=("k",)):
        piece()
    for piece in qk_pieces(0, tbs=(0,), which=("q",)):
        piece()

    emit_attention(0, uf0)
    emit_attention(1, uf1, unit_hooks=norm_hooks(0))
    emit_attention(2, uf2, unit_hooks=norm_hooks(1))
    emit_attention(3, uf3, unit_hooks={**norm_hooks(2), **hooks3})
    norm_qb(3, QB - 1)
    for piece in proj_pieces(3, QB - 1):
        piece()


_CACHED = None


def _get_nc():
    global _CACHED
    if _CACHED is None:
        nc = bass.Bass("TRN2", target_bir_lowering=False, debug=False)
        xT = nc.dram_tensor("xT", [C, N], BF16, kind="ExternalInput")
        wqT = nc.dram_tensor("wqT", [C, DQ], BF16, kind="ExternalInput")
        wkT = nc.dram_tensor("wkT", [C, DQ], BF16, kind="ExternalInput")
        wvT = nc.dram_tensor("wvT", [C, DQ], BF16, kind="ExternalInput")
        wpT = nc.dram_tensor("wpT", [DQ, C], BF16, kind="ExternalInput")
        outP = nc.dram_tensor("outP", [HC, C, N], F32, kind="ExternalOutput")
        with tile.TileContext(nc) as tc:
            with ExitStack() as ctx:
                _build(ctx, tc, xT, wqT, wkT, wvT, wpT, outP)
        _split_excess_waits(nc)
        _CACHED = nc
    return _CACHED


def run(x, mask, qkv_w, proj_w, proj_b, trace=False):
    x = np.asarray(x, dtype=np.float32)
    qkv_w = np.asarray(qkv_w, dtype=np.float32)
    proj_w = np.asarray(proj_w, dtype=np.float32)
    proj_b = np.asarray(proj_b, dtype=np.float32)

    in_maps = []
    for core in range(N_CORES):
        b, g = core // 2, core % 2
        r = slice(512 * g, 512 * g + 512)
        in_maps.append({
            "xT": np.ascontiguousarray(x[b].T).astype(NPBF16),
            "wqT": np.ascontiguousarray(qkv_w[r].T).astype(NPBF16),
            "wkT": np.ascontiguousarray(qkv_w[1024:][r].T).astype(NPBF16),
            "wvT": np.ascontiguousarray(qkv_w[2048:][r].T).astype(NPBF16),
            "wpT": np.ascontiguousarray(proj_w[:, r].T).astype(NPBF16),
        })

    nc = _get_nc()
    res = run_bass_kernel_spmd(
        nc, in_maps, core_ids=list(range(N_CORES)), trace=trace)

    out = np.empty((B, N, C), dtype=np.float32)
    for b in range(B):
        acc_np = (res.results[2 * b]["outP"].sum(axis=0)
                  + res.results[2 * b + 1]["outP"].sum(axis=0))
        out[b] = acc_np.T + proj_b
    return out, res


def kernel(x, mask, qkv_w, proj_w, proj_b):
    out, _ = run(x, mask, qkv_w, proj_w, proj_b, trace=False)
    return out


# revision 24
# speedup vs baseline: 1.3094x; 1.1656x over previous
"""Trainium2 Bass kernel for nn_Attention (B=4, N=2048, dim=1024, 16 heads).

Sharding: each of the 8 cores handles one (batch, head-group) pair —
batch b = core//2, head-group g = core%2 (8 heads each). Per core:
  qkv part  : Q^T,K^T = W_{q,k}[g] @ x_b^T (d-major), V natural
              (+ a ones column per head for the softmax1 denominator)
  attention : S^T = K^T-chunk.T @ Q^T-padded (full 128-row contraction;
              each head's Q^T lives in its own 128-row tile with the
              other head's rows zeroed, so every matmul in the kernel
              shares one PE config and stationary reloads stay hidden
              in the background weight buffer),
              P^T = exp(S^T * scale)  (no max subtraction: logits ~ N(0,1)),
              O^T[d,q] (+denom row) = [V|1].T @ P^T  accumulated over k,
              software-pipelined one k-chunk behind the exp stream
  normalize : recip = exp-free 1/(1+denom) on DVE, partition-broadcast
              via DRAM bounce, multiply
  proj      : OUT^T = Wp[g]-slices @ H^T   (partial over head channels)
Host side: per batch, out[b] = (OUT^T_{2b} + OUT^T_{2b+1}).T + proj_b.

All non-score PE work (QKV build, V build, projection) is emitted as
~2-matmul filler pieces spread evenly across the exp-paced units.
"""

import numpy as np
import ml_dtypes
from contextlib import ExitStack

import concourse.bass as bass
import concourse.tile as tile
from concourse import mybir
from concourse.bass_utils import run_bass_kernel_spmd

BF16 = mybir.dt.bfloat16
F32 = mybir.dt.float32
AF = mybir.ActivationFunctionType
NPBF16 = ml_dtypes.bfloat16

N_CORES = 8
B = 4
N = 2048          # tokens per batch
C = 1024          # model dim
NH = 8            # heads per core
HD = 64           # head dim
DQ = NH * HD      # q/k/v dims per core (512)
SCALE = HD ** -0.5
CC = C // 128     # contraction chunks (8)
QB = N // 512     # q blocks of 512 (4)
KC = N // 128     # k chunks of 128 (16)
HC = DQ // 128    # head pairs (4)
OT = C // 128     # output row tiles (8)
_MAX_WAITS = 1


def _split_excess_waits(nc):
    """This walrus build rejects >1 semaphore wait per instruction
    ("Too many sync wait commands"); move the excess onto NOPs inserted
    immediately before the offending instruction on the same engine."""
    n_new = 0
    for f in nc.m.functions:
        for bb in f.blocks:
            insts = bb.instructions
            i = 0
            while i < len(insts):
                inst = insts[i]
                si = inst.sync_info
                if si is not None and si.on_wait and len(si.on_wait) > _MAX_WAITS:
                    waits = list(si.on_wait)
                    keep, rest = waits[:_MAX_WAITS], waits[_MAX_WAITS:]
                    nops = []
                    while rest:
                        chunk, rest = rest[:_MAX_WAITS], rest[_MAX_WAITS:]
                        nop = mybir.InstNoOp(
                            name=f"wait-split-{n_new}", ins=[], outs=[])
                        n_new += 1
                        nop.engine = inst.engine
                        nop.sync_info = mybir.SyncInfo(on_wait=chunk, on_update=[])
                        nops.append(nop)
                    inst.sync_info = mybir.SyncInfo(
                        on_wait=keep, on_update=list(si.on_update or []))
                    for j, nop in enumerate(nops):
                        insts.insert(i + j, nop)
                    i += len(nops)
                i += 1
    return n_new


def _build(ctx: ExitStack, tc: tile.TileContext, xT, wqT, wkT, wvT, wpT, outP):
    nc = tc.nc

    persist = ctx.enter_context(tc.tile_pool(name="persist", bufs=1))
    p_pool = ctx.enter_context(tc.tile_pool(name="p", bufs=8))
    stg_pool = ctx.enter_context(tc.tile_pool(name="stg", bufs=3))
    den_pool = ctx.enter_context(tc.tile_pool(name="den2", bufs=2))
    dram_pool = ctx.enter_context(tc.tile_pool(name="scr", bufs=1, space="DRAM"))
    acc = ctx.enter_context(tc.tile_pool(name="acc", bufs=2, space="PSUM"))
    opair = ctx.enter_context(tc.tile_pool(name="opair", bufs=2, space="PSUM"))
    sc_pool = ctx.enter_context(tc.tile_pool(name="sc", bufs=2, space="PSUM"))

    wq = persist.tile([128, CC, DQ], BF16, tag="wq")
    wk = persist.tile([128, CC, DQ], BF16, tag="wk")
    wv = persist.tile([128, CC, DQ], BF16, tag="wv")
    wp = persist.tile([128, HC, C], BF16, tag="wp")

    # per-head-padded Q^T tiles: head hp of pair hc occupies rows
    # 64*hp..64*hp+63; the other 64 rows are zero so a full-128-row
    # matmul against the pair's K^T chunk yields that head's scores
    qTs = [[persist.tile([128, N], BF16, tag=f"qT{i}_{hp}",
                         name=f"qT{i}_{hp}") for hp in range(2)]
           for i in range(HC)]
    kTs = [persist.tile([128, N], BF16, tag=f"kT{i}", name=f"kT{i}")
           for i in range(HC)]
    vs = [persist.tile([128, NH * (HD + 1)], BF16, tag=f"v{i}", name=f"v{i}")
          for i in range(KC)]
    hT = persist.tile([128, HC, N], BF16, tag="hT")

    den_drams = [dram_pool.tile([2, N], F32, tag=f"dend{i}", name=f"dend{i}")
                 for i in range(HC)]
    recip_drams = [dram_pool.tile([2, N], F32, tag=f"recd{i}", name=f"recd{i}")
                   for i in range(HC)]

    # PE warmup first: ~4.5us of dummy matmuls so the HAM clock gate
    # opens (K=8/8, 2.4 GHz) before the real QKV matmuls start; without
    # this the whole head phase runs at 1.2 GHz
    warm = persist.tile([128, 512], BF16, tag="warm")
    nc.vector.memset(warm[:], 0.0)
    for i in range(24):
        wps = acc.tile([128, 512], F32, tag="acc", name="warmps")
        nc.tensor.matmul(wps[:], warm[:, 0:128], warm[:], start=True,
                         stop=True)

    # ones columns for the softmax1 denominator: strided memset touching
    # only the 8 ones columns per tile (a full-tile memset would cost
    # ~500ns each and serialize ahead of the first Q/K copies on DVE)
    for v_t in vs:
        nc.vector.memset(
            v_t[:].rearrange("p (h e) -> p h e", e=HD + 1)[:, :, HD:HD + 1],
            1.0)

    def emit_qpad(hc):
        # zero halves of pair hc's padded Q^T tiles (one-time)
        nc.vector.memset(qTs[hc][0][64:128, :], 0.0)
        nc.vector.memset(qTs[hc][1][0:64, :], 0.0)

    emit_qpad(0)

    oT_pool = ctx.enter_context(tc.tile_pool(name="oT", bufs=2))
    xt_pool = ctx.enter_context(tc.tile_pool(name="xt", bufs=1))
    rb_pool = ctx.enter_context(tc.tile_pool(name="rb", bufs=2))
    xts = [xt_pool.tile([128, N], BF16, tag=f"xt{i}", name=f"xt{i}")
           for i in range(CC)]

    # DMA order: wq, wk and x tb0 gate the first scores; wv next (V
    # fillers run during the first q-block); then the rest of x, then wp.
    # Weight descriptors go on the GpSimd software queue and x on the
    # Sync queue so their ~600ns dispatches overlap instead of
    # serializing on one engine.
    xT_r = xT.ap().rearrange("(cc p) t -> p cc t", p=128)
    wq_r = wqT.ap().rearrange("(cc p) d -> p cc d", p=128)
    wk_r = wkT.ap().rearrange("(cc p) d -> p cc d", p=128)
    wv_r = wvT.ap().rearrange("(cc p) d -> p cc d", p=128)
    for cc in range(0, CC, 2):
        nc.gpsimd.dma_start(
            out=wq[:, cc:cc + 2, :], in_=wq_r[:, cc:cc + 2, :])
        nc.gpsimd.dma_start(
            out=wk[:, cc:cc + 2, :], in_=wk_r[:, cc:cc + 2, :])
        nc.sync.dma_start(
            out=xts[cc][:, 0:512], in_=xT_r[:, cc, 0:512])
        nc.sync.dma_start(
            out=xts[cc + 1][:, 0:512], in_=xT_r[:, cc + 1, 0:512])
    for cc in range(0, CC, 2):
        nc.gpsimd.dma_start(
            out=wv[:, cc:cc + 2, :], in_=wv_r[:, cc:cc + 2, :])
    for tb in range(1, QB):
        for cc in range(CC):
            nc.sync.dma_start(
                out=xts[cc][:, tb * 512:(tb + 1) * 512],
                in_=xT_r[:, cc, tb * 512:(tb + 1) * 512])
    nc.gpsimd.dma_start(
        out=wp[:], in_=wpT.ap().rearrange("(hc p) o -> p hc o", p=128))

    # ---- filler piece machinery (all matmuls share the 128x128 config,
    # each piece <= ~2 matmuls so no unit overloads the PE) -------------

    def qk_pieces(hc, tbs=None, which=("q", "k")):
        for tb in (range(QB) if tbs is None else tbs):
            for w in which:
                w_sb = wq if w == "q" else wk
                state = {}

                def mk(cc0, w=w, w_sb=w_sb, tb=tb, state=state, hc=hc):
                    def piece():
                        if cc0 == 0:
                            state["ps"] = acc.tile(
                                [128, 512], F32, tag="acc", name="qkps")
                        ps = state["ps"]
                        for cc in (cc0, cc0 + 1):
                            nc.tensor.matmul(
                                ps[:],
                                w_sb[:, cc, hc * 128:(hc + 1) * 128],
                                xts[cc][:, tb * 512:(tb + 1) * 512],
                                start=(cc == 0), stop=(cc == CC - 1))
                        if cc0 == CC - 2:
                            ts = slice(tb * 512, (tb + 1) * 512)
                            if w == "q":
                                nc.vector.tensor_copy(
                                    qTs[hc][0][0:64, ts], ps[0:64, :])
                                nc.vector.tensor_copy(
                                    qTs[hc][1][64:128, ts], ps[64:128, :])
                            else:
                                nc.vector.tensor_copy(kTs[hc][:, ts], ps[:])
                    return piece
                for cc0 in range(0, CC, 2):
                    yield mk(cc0)

    def v_pieces(tci):
        state = {}

        def mk(cc0):
            def piece():
                if cc0 == 0:
                    state["ps"] = acc.tile(
                        [128, 512], F32, tag="acc", name="vps")
                ps = state["ps"]
                for cc in (cc0, cc0 + 1):
                    nc.tensor.matmul(
                        ps[:],
                        xts[cc][:, tci * 128:(tci + 1) * 128],
                        wv[:, cc, :],
                        start=(cc == 0), stop=(cc == CC - 1))
                if cc0 == CC - 2:
                    nc.vector.tensor_copy(
                        vs[tci][:].rearrange(
                            "p (h e) -> p h e", e=HD + 1)[:, :, 0:HD],
                        ps[:].rearrange("p (h e) -> p h e", e=HD))
            return piece
        for cc0 in range(0, CC, 2):
            yield mk(cc0)

    def proj_pieces(tb):
        # full projection for token block tb: the four head-pair partials
        # accumulate in PSUM (4 matmuls), then one copy + one store —
        # quarter the DVE-copy and output-DMA traffic of per-pair partials
        outP_r = outP.ap().rearrange("(ot p) t -> p ot t", p=128)
        for ot in range(OT):
            def piece(ot=ot):
                ps = acc.tile([128, 512], F32, tag="acc", name="prps")
                for hc in range(HC):
                    nc.tensor.matmul(
                        ps[:],
                        wp[:, hc, ot * 128:(ot + 1) * 128],
                        hT[:, hc, tb * 512:(tb + 1) * 512],
                        start=(hc == 0), stop=(hc == HC - 1))
                so = stg_pool.tile([128, 512], F32, tag="stg", name="so")
                nc.vector.tensor_copy(so[:], ps[:])
                nc.sync.dma_start(
                    out=outP_r[:, ot, tb * 512:(tb + 1) * 512], in_=so[:])
            yield piece

    oT_tiles = {}

    def norm_qb(hc, qb):
        # recip = 1/(1+den) for one q-block of pair hc (DVE + DMA only;
        # no PE work). den rows were extracted to DRAM by emit_attention.
        qs = slice(qb * 512, (qb + 1) * 512)
        den2 = den_pool.tile([128, 8], F32, tag="den2", name="den2")
        nc.gpsimd.dma_start(
            out=den2[:],
            in_=den_drams[hc][:, qs].rearrange("h (a i) -> h a i", i=8))
        nc.vector.tensor_scalar_add(den2[:], den2[:], 1.0)
        nc.vector.reciprocal(den2[:], den2[:])
        nc.gpsimd.dma_start(
            out=recip_drams[hc][:, qs].rearrange("h (a i) -> h a i", i=8),
            in_=den2[:])
        rb_t = rb_pool.tile([128, 512], F32, tag="rb", name="rb")
        for half in range(2):
            src = recip_drams[hc][half:half + 1, qs].broadcast_to((64, 512))
            nc.gpsimd.dma_start(out=rb_t[half * 64:(half + 1) * 64, :], in_=src)
        nc.vector.tensor_mul(
            hT[:, hc, qs], oT_tiles[hc][:, qs], rb_t[:])

    def emit_attention(hc, unit_fillers, unit_hooks=None):
        """unit_fillers[qb*KC+kc]: filler pieces to run in that unit.
        unit_hooks: optional dict {unit_index: callable} for norm/proj
        staggering of the final pair."""
        oT_t = oT_pool.tile([128, N], F32, tag="oT", name=f"oT{hc}")
        oT_tiles[hc] = oT_t
        vcols = [(2 * hc + hp) * (HD + 1) for hp in range(2)]
        for qb in range(QB):
            qs = slice(qb * 512, (qb + 1) * 512)
            o_ps = [opair.tile([128, 512], F32, tag="opair", name=f"ops{hp}")
                    for hp in range(2)]

            def attn_chunk(kc, p_sb):
                for hp in range(2):
                    nc.tensor.matmul(
                        o_ps[hp][0:HD + 1, :],
                        vs[kc][:, vcols[hp]:vcols[hp] + HD + 1],
                        p_sb[:, hp, :],
                        start=(kc == 0), stop=(kc == KC - 1))

            # software pipeline depth 2: attnV runs two chunks behind the
            # exp stream, so at q-block boundaries the o_ps bank reuse
            # (gated on the previous block's drain copies) never blocks
            # the next scores in the in-order PE queue
            pend = []
            for kc in range(KC):
                u = qb * KC + kc
                if unit_hooks and u in unit_hooks:
                    unit_hooks[u]()
                # scores: full-128-row matmuls against the padded Q^T
                # tiles — same PE config as every other matmul here
                s_ps = sc_pool.tile([128, 2, 512], F32, tag="sc")
                for hp in range(2):
                    nc.tensor.matmul(
                        s_ps[:, hp, :],
                        kTs[hc][:, kc * 128:(kc + 1) * 128],
                        qTs[hc][hp][:, qs],
                        start=True, stop=True)
                p_sb = p_pool.tile([128, 2, 512], BF16, tag="p")
                nc.scalar.activation(
                    out=p_sb[:], in_=s_ps[:], func=AF.Exp, scale=SCALE)
                pend.append((kc, p_sb))
                if len(pend) > 2:
                    attn_chunk(*pend.pop(0))
                for piece in unit_fillers[u]:
                    piece()
            for item in pend:
                attn_chunk(*item)

            # drain O^T + denominator rows (baseline mechanics: head 0
            # lands in place; head 1 needs a partition shift via DMA)
            for hp in range(2):
                stg = stg_pool.tile([128, 512], F32, tag="stg")
                if hp == 0:
                    nc.vector.tensor_copy(oT_t[0:HD, qs], o_ps[0][0:HD, :])
                    nc.vector.tensor_copy(
                        stg[HD:HD + 1, :], o_ps[0][HD:HD + 1, :])
                else:
                    nc.vector.tensor_copy(
                        stg[0:HD + 1, :], o_ps[1][0:HD + 1, :])
                    nc.sync.dma_start(
                        out=oT_t[HD:2 * HD, qs], in_=stg[0:HD, :])
                nc.gpsimd.dma_start(
                    out=den_drams[hc][hp:hp + 1, qs], in_=stg[HD:HD + 1, :])

    # ---- static filler schedule ------------------------------------------
    def spread(units, pieces):
        pieces = list(pieces)
        if not pieces:
            return
        for i, piece in enumerate(pieces):
            units[i * len(units) // len(pieces)].append(piece)

    def unit_lists():
        return [[] for _ in range(QB * KC)]

    # pair 0: qb0 must build V (one chunk per unit, just ahead of its
    # first consumer — attnV runs 2 behind, so chunk kc is due at unit
    # kc+2); each q-block also builds the next q-block's Q tile; qb1-3
    # carry the K/Q build of pair 1 (minus the k-parts deferred to
    # pair 1's own early units)
    uf0 = unit_lists()
    for kc in range(KC):
        pieces = list(v_pieces(kc))
        slots = [max(kc - 1, 0), min(kc + 1, KC - 1)]
        for i, piece in enumerate(pieces):
            uf0[slots[i * len(slots) // len(pieces)]].append(piece)
    for tb in range(1, QB):
        spread([uf0[u] for u in range((tb - 1) * KC + 8, tb * KC)],
               qk_pieces(0, tbs=(tb,), which=("q",)))
    spread([uf0[u] for u in range(KC, QB * KC)],
           list(qk_pieces(1, which=("q",)))
           + list(qk_pieces(1, tbs=(0, 1), which=("k",))))
    uf0[KC].append(lambda: emit_qpad(1))

    # pair 1: rest of pair 1's K (its chunks kc>=8 are consumed from
    # unit 8 on), K/Q of pair 2; pair 2: K/Q of pair 3
    uf1 = unit_lists()
    spread([uf1[u] for u in range(0, 8)],
           qk_pieces(1, tbs=(2, 3), which=("k",)))
    spread([uf1[u] for u in range(8, QB * KC, 2)], qk_pieces(2))
    uf1[KC].append(lambda: emit_qpad(2))
    uf2 = unit_lists()
    spread([uf2[u] for u in range(0, QB * KC, 2)], qk_pieces(3))
    uf2[KC].append(lambda: emit_qpad(3))

    # pair 3 carries the (cross-pair accumulated) projection: token
    # block tb is ready once norm(3, tb) ran; its norm fires one unit
    # into the next q-block and the proj pieces trail 7+ units behind
    # so their PSUM->copy->store chains never head-of-line block scores
    uf3 = unit_lists()
    hooks3 = {}
    for qb in range(1, QB):
        hooks3[qb * KC + 1] = (lambda qb=qb: norm_qb(3, qb - 1))
        for i, piece in enumerate(proj_pieces(qb - 1)):
            hooks3[qb * KC + 8 + i] = piece

    # norms for pairs 0-2 run at the start of the NEXT pair's stream
    # (hT[hc] must be ready before pair 3's proj); they cost no PE
    def norm_hooks(hc):
        hooks = {}
        for qb in range(QB):
            def hook(hc=hc, qb=qb):
                norm_qb(hc, qb)
            hooks[qb * 4 + 2] = hook  # early units of the next pass
        return hooks

    # ---- emission ---------------------------------------------------------
    # minimal head: the full K tile of pair 0 (scores chunk kc needs K
    # columns kc*128.. as the stream advances) and q-block 0's Q tile;
    # the other Q blocks are built as fillers one q-block ahead
    for piece in qk_pieces(0, tbs=(0, 1, 2, 3), which=("k",)):
        piece()
    for piece in qk_pieces(0, tbs=(0,), which=("q",)):
        piece()

    emit_attention(0, uf0)
    emit_attention(1, uf1, unit_hooks=norm_hooks(0))
    emit_attention(2, uf2, unit_hooks=norm_hooks(1))
    emit_attention(3, uf3, unit_hooks={**norm_hooks(2), **hooks3})
    norm_qb(3, QB - 1)
    for piece in proj_pieces(QB - 1):
        piece()


_CACHED = None


def _get_nc():
    global _CACHED
    if _CACHED is None:
        nc = bass.Bass("TRN2", target_bir_lowering=False, debug=False)
        xT = nc.dram_tensor("xT", [C, N], BF16, kind="ExternalInput")
        wqT = nc.dram_tensor("wqT", [C, DQ], BF16, kind="ExternalInput")
        wkT = nc.dram_tensor("wkT", [C, DQ], BF16, kind="ExternalInput")
        wvT = nc.dram_tensor("wvT", [C, DQ], BF16, kind="ExternalInput")
        wpT = nc.dram_tensor("wpT", [DQ, C], BF16, kind="ExternalInput")
        outP = nc.dram_tensor("outP", [C, N], F32, kind="ExternalOutput")
        with tile.TileContext(nc) as tc:
            with ExitStack() as ctx:
                _build(ctx, tc, xT, wqT, wkT, wvT, wpT, outP)
        _split_excess_waits(nc)
        _CACHED = nc
    return _CACHED


def run(x, mask, qkv_w, proj_w, proj_b, trace=False):
    x = np.asarray(x, dtype=np.float32)
    qkv_w = np.asarray(qkv_w, dtype=np.float32)
    proj_w = np.asarray(proj_w, dtype=np.float32)
    proj_b = np.asarray(proj_b, dtype=np.float32)

    in_maps = []
    for core in range(N_CORES):
        b, g = core // 2, core % 2
        r = slice(512 * g, 512 * g + 512)
        in_maps.append({
            "xT": np.ascontiguousarray(x[b].T).astype(NPBF16),
            "wqT": np.ascontiguousarray(qkv_w[r].T).astype(NPBF16),
            "wkT": np.ascontiguousarray(qkv_w[1024:][r].T).astype(NPBF16),
            "wvT": np.ascontiguousarray(qkv_w[2048:][r].T).astype(NPBF16),
            "wpT": np.ascontiguousarray(proj_w[:, r].T).astype(NPBF16),
        })

    nc = _get_nc()
    res = run_bass_kernel_spmd(
        nc, in_maps, core_ids=list(range(N_CORES)), trace=trace)

    out = np.empty((B, N, C), dtype=np.float32)
    for b in range(B):
        acc_np = res.results[2 * b]["outP"] + res.results[2 * b + 1]["outP"]
        out[b] = acc_np.T + proj_b
    return out, res


def kernel(x, mask, qkv_w, proj_w, proj_b):
    out, _ = run(x, mask, qkv_w, proj_w, proj_b, trace=False)
    return out


# revision 31
# speedup vs baseline: 1.3100x; 1.0005x over previous
"""Trainium2 Bass kernel for nn_Attention (B=4, N=2048, dim=1024, 16 heads).

Sharding: each of the 8 cores handles one (batch, head-group) pair —
batch b = core//2, head-group g = core%2 (8 heads each). Per core:
  qkv part  : Q^T,K^T = W_{q,k}[g] @ x_b^T (d-major), V natural
              (+ a ones column per head for the softmax1 denominator)
  attention : S^T = K^T-chunk.T @ Q^T-padded (full 128-row contraction;
              each head's Q^T lives in its own 128-row tile with the
              other head's rows zeroed, so every matmul in the kernel
              shares one PE config and stationary reloads stay hidden
              in the background weight buffer),
              P^T = exp(S^T * scale)  (no max subtraction: logits ~ N(0,1)),
              O^T[d,q] (+denom row) = [V|1].T @ P^T  accumulated over k,
              software-pipelined one k-chunk behind the exp stream
  normalize : recip = exp-free 1/(1+denom) on DVE, partition-broadcast
              via DRAM bounce, multiply
  proj      : OUT^T = Wp[g]-slices @ H^T   (partial over head channels)
Host side: per batch, out[b] = (OUT^T_{2b} + OUT^T_{2b+1}).T + proj_b.

All non-score PE work (QKV build, V build, projection) is emitted as
~2-matmul filler pieces spread evenly across the exp-paced units.
"""

import numpy as np
import ml_dtypes
from contextlib import ExitStack

import concourse.bass as bass
import concourse.tile as tile
from concourse import mybir
from concourse.bass_utils import run_bass_kernel_spmd

BF16 = mybir.dt.bfloat16
F32 = mybir.dt.float32
AF = mybir.ActivationFunctionType
NPBF16 = ml_dtypes.bfloat16

N_CORES = 8
B = 4
N = 2048          # tokens per batch
C = 1024          # model dim
NH = 8            # heads per core
HD = 64           # head dim
DQ = NH * HD      # q/k/v dims per core (512)
SCALE = HD ** -0.5
CC = C // 128     # contraction chunks (8)
QB = N // 512     # q blocks of 512 (4)
KC = N // 128     # k chunks of 128 (16)
HC = DQ // 128    # head pairs (4)
OT = C // 128     # output row tiles (8)
_MAX_WAITS = 1


def _split_excess_waits(nc):
    """This walrus build rejects >1 semaphore wait per instruction
    ("Too many sync wait commands"); move the excess onto NOPs inserted
    immediately before the offending instruction on the same engine."""
    n_new = 0
    for f in nc.m.functions:
        for bb in f.blocks:
            insts = bb.instructions
            i = 0
            while i < len(insts):
                inst = insts[i]
                si = inst.sync_info
                if si is not None and si.on_wait and len(si.on_wait) > _MAX_WAITS:
                    waits = list(si.on_wait)
                    keep, rest = waits[:_MAX_WAITS], waits[_MAX_WAITS:]
                    nops = []
                    while rest:
                        chunk, rest = rest[:_MAX_WAITS], rest[_MAX_WAITS:]
                        nop = mybir.InstNoOp(
                            name=f"wait-split-{n_new}", ins=[], outs=[])
                        n_new += 1
                        nop.engine = inst.engine
                        nop.sync_info = mybir.SyncInfo(on_wait=chunk, on_update=[])
                        nops.append(nop)
                    inst.sync_info = mybir.SyncInfo(
                        on_wait=keep, on_update=list(si.on_update or []))
                    for j, nop in enumerate(nops):
                        insts.insert(i + j, nop)
                    i += len(nops)
                i += 1
    return n_new


def _build(ctx: ExitStack, tc: tile.TileContext, xT, wqT, wkT, wvT, wpT, outP):
    nc = tc.nc

    persist = ctx.enter_context(tc.tile_pool(name="persist", bufs=1))
    p_pool = ctx.enter_context(tc.tile_pool(name="p", bufs=8))
    stg_pool = ctx.enter_context(tc.tile_pool(name="stg", bufs=3))
    dram_pool = ctx.enter_context(tc.tile_pool(name="scr", bufs=1, space="DRAM"))
    acc = ctx.enter_context(tc.tile_pool(name="acc", bufs=2, space="PSUM"))
    opair = ctx.enter_context(tc.tile_pool(name="opair", bufs=2, space="PSUM"))
    sc_pool = ctx.enter_context(tc.tile_pool(name="sc", bufs=2, space="PSUM"))

    wq = persist.tile([128, CC, DQ], BF16, tag="wq")
    wk = persist.tile([128, CC, DQ], BF16, tag="wk")
    wv = persist.tile([128, CC, DQ], BF16, tag="wv")
    wp = persist.tile([128, HC, C], BF16, tag="wp")

    # per-head-padded Q^T tiles: head hp of pair hc occupies rows
    # 64*hp..64*hp+63; the other 64 rows are zero so a full-128-row
    # matmul against the pair's K^T chunk yields that head's scores
    qTs = [[persist.tile([128, N], BF16, tag=f"qT{i}_{hp}",
                         name=f"qT{i}_{hp}") for hp in range(2)]
           for i in range(HC)]
    kTs = [persist.tile([128, N], BF16, tag=f"kT{i}", name=f"kT{i}")
           for i in range(HC)]
    vs = [persist.tile([128, NH * (HD + 1)], BF16, tag=f"v{i}", name=f"v{i}")
          for i in range(KC)]
    hT = persist.tile([128, HC, N], BF16, tag="hT")

    recip_drams = [dram_pool.tile([2, N], F32, tag=f"recd{i}", name=f"recd{i}")
                   for i in range(HC)]

    # PE warmup first: ~4.5us of dummy matmuls so the HAM clock gate
    # opens (K=8/8, 2.4 GHz) before the real QKV matmuls start; without
    # this the whole head phase runs at 1.2 GHz
    warm = persist.tile([128, 512], BF16, tag="warm")
    nc.vector.memset(warm[:], 0.0)
    for i in range(24):
        wps = acc.tile([128, 512], F32, tag="acc", name="warmps")
        nc.tensor.matmul(wps[:], warm[:, 0:128], warm[:], start=True,
                         stop=True)

    # ones columns for the softmax1 denominator: strided memset touching
    # only the 8 ones columns per tile (a full-tile memset would cost
    # ~500ns each and serialize ahead of the first Q/K copies on DVE)
    for v_t in vs:
        nc.vector.memset(
            v_t[:].rearrange("p (h e) -> p h e", e=HD + 1)[:, :, HD:HD + 1],
            1.0)

    def emit_qpad(hc):
        # zero halves of pair hc's padded Q^T tiles (one-time)
        nc.vector.memset(qTs[hc][0][64:128, :], 0.0)
        nc.vector.memset(qTs[hc][1][0:64, :], 0.0)

    emit_qpad(0)

    oT_pool = ctx.enter_context(tc.tile_pool(name="oT", bufs=2))
    xt_pool = ctx.enter_context(tc.tile_pool(name="xt", bufs=1))
    rb_pool = ctx.enter_context(tc.tile_pool(name="rb", bufs=2))
    xts = [xt_pool.tile([128, N], BF16, tag=f"xt{i}", name=f"xt{i}")
           for i in range(CC)]

    # DMA: a single queue moves only ~70 GB/s, so spread the input loads
    # over all three DMA-capable queues — x on Sync + Scalar (the ACT
    # HW-DGE is free until the exp stream begins), weights on the GpSimd
    # software queue. wq, wk and x tb0 gate the first scores.
    xT_r = xT.ap().rearrange("(cc p) t -> p cc t", p=128)
    wq_r = wqT.ap().rearrange("(cc p) d -> p cc d", p=128)
    wk_r = wkT.ap().rearrange("(cc p) d -> p cc d", p=128)
    wv_r = wvT.ap().rearrange("(cc p) d -> p cc d", p=128)
    for cc in range(0, CC, 2):
        nc.gpsimd.dma_start(
            out=wq[:, cc:cc + 2, :], in_=wq_r[:, cc:cc + 2, :])
        nc.gpsimd.dma_start(
            out=wk[:, cc:cc + 2, :], in_=wk_r[:, cc:cc + 2, :])
        nc.sync.dma_start(
            out=xts[cc][:, 0:512], in_=xT_r[:, cc, 0:512])
        nc.scalar.dma_start(
            out=xts[cc + 1][:, 0:512], in_=xT_r[:, cc + 1, 0:512])
    for cc in range(0, CC, 2):
        nc.gpsimd.dma_start(
            out=wv[:, cc:cc + 2, :], in_=wv_r[:, cc:cc + 2, :])
    for tb in range(1, QB):
        for cc in range(CC):
            eng = nc.sync if cc % 2 == 0 else nc.scalar
            eng.dma_start(
                out=xts[cc][:, tb * 512:(tb + 1) * 512],
                in_=xT_r[:, cc, tb * 512:(tb + 1) * 512])
    nc.gpsimd.dma_start(
        out=wp[:], in_=wpT.ap().rearrange("(hc p) o -> p hc o", p=128))

    # ---- filler piece machinery (all matmuls share the 128x128 config,
    # each piece <= ~2 matmuls so no unit overloads the PE) -------------

    def qk_pieces(hc, tbs=None, which=("q", "k")):
        for tb in (range(QB) if tbs is None else tbs):
            for w in which:
                w_sb = wq if w == "q" else wk
                state = {}

                def mk(cc0, w=w, w_sb=w_sb, tb=tb, state=state, hc=hc):
                    def piece():
                        if cc0 == 0:
                            state["ps"] = acc.tile(
                                [128, 512], F32, tag="acc", name="qkps")
                        ps = state["ps"]
                        for cc in (cc0, cc0 + 1):
                            nc.tensor.matmul(
                                ps[:],
                                w_sb[:, cc, hc * 128:(hc + 1) * 128],
                                xts[cc][:, tb * 512:(tb + 1) * 512],
                                start=(cc == 0), stop=(cc == CC - 1))
                        if cc0 == CC - 2:
                            ts = slice(tb * 512, (tb + 1) * 512)
                            if w == "q":
                                nc.vector.tensor_copy(
                                    qTs[hc][0][0:64, ts], ps[0:64, :])
                                nc.vector.tensor_copy(
                                    qTs[hc][1][64:128, ts], ps[64:128, :])
                            else:
                                nc.vector.tensor_copy(kTs[hc][:, ts], ps[:])
                    return piece
                for cc0 in range(0, CC, 2):
                    yield mk(cc0)

    def v_pieces(tci):
        state = {}

        def mk(cc0):
            def piece():
                if cc0 == 0:
                    state["ps"] = acc.tile(
                        [128, 512], F32, tag="acc", name="vps")
                ps = state["ps"]
                for cc in (cc0, cc0 + 1):
                    nc.tensor.matmul(
                        ps[:],
                        xts[cc][:, tci * 128:(tci + 1) * 128],
                        wv[:, cc, :],
                        start=(cc == 0), stop=(cc == CC - 1))
                if cc0 == CC - 2:
                    nc.vector.tensor_copy(
                        vs[tci][:].rearrange(
                            "p (h e) -> p h e", e=HD + 1)[:, :, 0:HD],
                        ps[:].rearrange("p (h e) -> p h e", e=HD))
            return piece
        for cc0 in range(0, CC, 2):
            yield mk(cc0)

    def proj_pieces(tb):
        # full projection for token block tb: the four head-pair partials
        # accumulate in PSUM (4 matmuls), then one copy + one store —
        # quarter the DVE-copy and output-DMA traffic of per-pair partials
        outP_r = outP.ap().rearrange("(ot p) t -> p ot t", p=128)
        for ot in range(OT):
            def piece(ot=ot):
                ps = acc.tile([128, 512], F32, tag="acc", name="prps")
                for hc in range(HC):
                    nc.tensor.matmul(
                        ps[:],
                        wp[:, hc, ot * 128:(ot + 1) * 128],
                        hT[:, hc, tb * 512:(tb + 1) * 512],
                        start=(hc == 0), stop=(hc == HC - 1))
                so = stg_pool.tile([128, 512], F32, tag="stg", name="so")
                nc.vector.tensor_copy(so[:], ps[:])
                nc.sync.dma_start(
                    out=outP_r[:, ot, tb * 512:(tb + 1) * 512], in_=so[:])
            yield piece

    oT_tiles = {}

    def norm_qb(hc, qb):
        # multiply O^T by the reciprocals (computed on the drain path and
        # already staged in DRAM): 2 broadcast DMAs + one DVE multiply
        qs = slice(qb * 512, (qb + 1) * 512)
        rb_t = rb_pool.tile([128, 512], F32, tag="rb", name="rb")
        for half in range(2):
            src = recip_drams[hc][half:half + 1, qs].broadcast_to((64, 512))
            nc.sync.dma_start(out=rb_t[half * 64:(half + 1) * 64, :], in_=src)
        nc.vector.tensor_mul(
            hT[:, hc, qs], oT_tiles[hc][:, qs], rb_t[:])

    def emit_attention(hc, unit_fillers, unit_hooks=None):
        """unit_fillers[qb*KC+kc]: filler pieces to run in that unit.
        unit_hooks: optional dict {unit_index: callable} for norm/proj
        staggering of the final pair."""
        oT_t = oT_pool.tile([128, N], F32, tag="oT", name=f"oT{hc}")
        oT_tiles[hc] = oT_t
        vcols = [(2 * hc + hp) * (HD + 1) for hp in range(2)]
        for qb in range(QB):
            qs = slice(qb * 512, (qb + 1) * 512)
            o_ps = [opair.tile([128, 512], F32, tag="opair", name=f"ops{hp}")
                    for hp in range(2)]

            def attn_chunk(kc, p_sb):
                for hp in range(2):
                    nc.tensor.matmul(
                        o_ps[hp][0:HD + 1, :],
                        vs[kc][:, vcols[hp]:vcols[hp] + HD + 1],
                        p_sb[:, hp, :],
                        start=(kc == 0), stop=(kc == KC - 1))

            # software pipeline depth 2: attnV runs two chunks behind the
            # exp stream, so at q-block boundaries the o_ps bank reuse
            # (gated on the previous block's drain copies) never blocks
            # the next scores in the in-order PE queue
            pend = []
            for kc in range(KC):
                u = qb * KC + kc
                if unit_hooks and u in unit_hooks:
                    unit_hooks[u]()
                # scores: full-128-row matmuls against the padded Q^T
                # tiles — same PE config as every other matmul here
                s_ps = sc_pool.tile([128, 2, 512], F32, tag="sc")
                for hp in range(2):
                    nc.tensor.matmul(
                        s_ps[:, hp, :],
                        kTs[hc][:, kc * 128:(kc + 1) * 128],
                        qTs[hc][hp][:, qs],
                        start=True, stop=True)
                p_sb = p_pool.tile([128, 2, 512], BF16, tag="p")
                nc.scalar.activation(
                    out=p_sb[:], in_=s_ps[:], func=AF.Exp, scale=SCALE)
                pend.append((kc, p_sb))
                if len(pend) > 2:
                    attn_chunk(*pend.pop(0))
                for piece in unit_fillers[u]:
                    piece()
            for item in pend:
                attn_chunk(*item)

            # drain O^T + denominator rows (head 0 lands in place; head 1
            # needs a partition shift via DMA). The softmax1 reciprocal
            # 1/(1+den) is computed right here on the staged den row, so
            # norm_qb later is just a broadcast DMA + multiply.
            for hp in range(2):
                stg = stg_pool.tile([128, 512], F32, tag="stg")
                if hp == 0:
                    nc.vector.tensor_copy(oT_t[0:HD, qs], o_ps[0][0:HD, :])
                    nc.vector.tensor_copy(
                        stg[HD:HD + 1, :], o_ps[0][HD:HD + 1, :])
                else:
                    nc.vector.tensor_copy(
                        stg[0:HD + 1, :], o_ps[1][0:HD + 1, :])
                    nc.sync.dma_start(
                        out=oT_t[HD:2 * HD, qs], in_=stg[0:HD, :])
                nc.vector.tensor_scalar_add(
                    stg[HD:HD + 1, :], stg[HD:HD + 1, :], 1.0)
                nc.vector.reciprocal(stg[HD:HD + 1, :], stg[HD:HD + 1, :])
                nc.gpsimd.dma_start(
                    out=recip_drams[hc][hp:hp + 1, qs],
                    in_=stg[HD:HD + 1, :])

    # ---- static filler schedule ------------------------------------------
    def spread(units, pieces):
        pieces = list(pieces)
        if not pieces:
            return
        for i, piece in enumerate(pieces):
            units[i * len(units) // len(pieces)].append(piece)

    def unit_lists():
        return [[] for _ in range(QB * KC)]

    # pair 0: qb0 must build V (one chunk per unit, just ahead of its
    # first consumer — attnV runs 2 behind, so chunk kc is due at unit
    # kc+2); each q-block also builds the next q-block's Q tile; qb1-3
    # carry the K/Q build of pair 1 (minus the k-parts deferred to
    # pair 1's own early units)
    uf0 = unit_lists()
    # pair 0's own K tiles for token blocks 1-3 stream in just ahead of
    # the score chunks that consume them (chunk 4j needs block j's K),
    # right behind the corresponding x DMA
    for tb in range(1, QB):
        spread([uf0[u] for u in range((tb - 1) * 4, tb * 4 - 1)],
               qk_pieces(0, tbs=(tb,), which=("k",)))
    for kc in range(KC):
        pieces = list(v_pieces(kc))
        slots = [max(kc - 1, 0), min(kc + 1, KC - 1)]
        for i, piece in enumerate(pieces):
            uf0[slots[i * len(slots) // len(pieces)]].append(piece)
    for tb in range(1, QB):
        spread([uf0[u] for u in range((tb - 1) * KC + 8, tb * KC)],
               qk_pieces(0, tbs=(tb,), which=("q",)))
    spread([uf0[u] for u in range(KC, QB * KC)],
           list(qk_pieces(1, which=("q",)))
           + list(qk_pieces(1, tbs=(0, 1), which=("k",))))
    uf0[KC].append(lambda: emit_qpad(1))

    # pair 1: rest of pair 1's K (its chunks kc>=8 are consumed from
    # unit 8 on), K/Q of pair 2; pair 2: K/Q of pair 3
    uf1 = unit_lists()
    spread([uf1[u] for u in range(0, 8)],
           qk_pieces(1, tbs=(2, 3), which=("k",)))
    spread([uf1[u] for u in range(8, QB * KC, 2)], qk_pieces(2))
    uf1[KC].append(lambda: emit_qpad(2))
    uf2 = unit_lists()
    spread([uf2[u] for u in range(0, QB * KC, 2)], qk_pieces(3))
    uf2[KC].append(lambda: emit_qpad(3))

    # pair 3 carries the (cross-pair accumulated) projection: token
    # block tb is ready once norm(3, tb) ran; its norm fires one unit
    # into the next q-block and the proj pieces trail 7+ units behind
    # so their PSUM->copy->store chains never head-of-line block scores
    uf3 = unit_lists()
    hooks3 = {}
    for qb in range(1, QB):
        hooks3[qb * KC + 1] = (lambda qb=qb: norm_qb(3, qb - 1))
        for i, piece in enumerate(proj_pieces(qb - 1)):
            hooks3[qb * KC + 8 + i] = piece

    # norms for pairs 0-2 run at the start of the NEXT pair's stream
    # (hT[hc] must be ready before pair 3's proj); they cost no PE
    def norm_hooks(hc):
        hooks = {}
        for qb in range(QB):
            def hook(hc=hc, qb=qb):
                norm_qb(hc, qb)
            hooks[qb * 4 + 2] = hook  # early units of the next pass
        return hooks

    # ---- emission ---------------------------------------------------------
    # minimal head: only what the first score chunks need — K and Q of
    # token block 0 (the other K blocks stream as early fillers above)
    for piece in qk_pieces(0, tbs=(0,), which=("k",)):
        piece()
    for piece in qk_pieces(0, tbs=(0,), which=("q",)):
        piece()

    emit_attention(0, uf0)
    emit_attention(1, uf1, unit_hooks=norm_hooks(0))
    emit_attention(2, uf2, unit_hooks=norm_hooks(1))
    emit_attention(3, uf3, unit_hooks={**norm_hooks(2), **hooks3})
    norm_qb(3, QB - 1)
    for piece in proj_pieces(QB - 1):
        piece()


_CACHED = None


def _get_nc():
    global _CACHED
    if _CACHED is None:
        nc = bass.Bass("TRN2", target_bir_lowering=False, debug=False)
        xT = nc.dram_tensor("xT", [C, N], BF16, kind="ExternalInput")
        wqT = nc.dram_tensor("wqT", [C, DQ], BF16, kind="ExternalInput")
        wkT = nc.dram_tensor("wkT", [C, DQ], BF16, kind="ExternalInput")
        wvT = nc.dram_tensor("wvT", [C, DQ], BF16, kind="ExternalInput")
        wpT = nc.dram_tensor("wpT", [DQ, C], BF16, kind="ExternalInput")
        outP = nc.dram_tensor("outP", [C, N], F32, kind="ExternalOutput")
        with tile.TileContext(nc) as tc:
            with ExitStack() as ctx:
                _build(ctx, tc, xT, wqT, wkT, wvT, wpT, outP)
        _split_excess_waits(nc)
        _CACHED = nc
    return _CACHED


def run(x, mask, qkv_w, proj_w, proj_b, trace=False):
    x = np.asarray(x, dtype=np.float32)
    qkv_w = np.asarray(qkv_w, dtype=np.float32)
    proj_w = np.asarray(proj_w, dtype=np.float32)
    proj_b = np.asarray(proj_b, dtype=np.float32)

    in_maps = []
    for core in range(N_CORES):
        b, g = core // 2, core % 2
        r = slice(512 * g, 512 * g + 512)
        in_maps.append({
            "xT": np.ascontiguousarray(x[b].T).astype(NPBF16),
            "wqT": np.ascontiguousarray(qkv_w[r].T).astype(NPBF16),
            "wkT": np.ascontiguousarray(qkv_w[1024:][r].T).astype(NPBF16),
            "wvT": np.ascontiguousarray(qkv_w[2048:][r].T).astype(NPBF16),
            "wpT": np.ascontiguousarray(proj_w[:, r].T).astype(NPBF16),
        })

    nc = _get_nc()
    res = run_bass_kernel_spmd(
        nc, in_maps, core_ids=list(range(N_CORES)), trace=trace)

    out = np.empty((B, N, C), dtype=np.float32)
    for b in range(B):
        acc_np = res.results[2 * b]["outP"] + res.results[2 * b + 1]["outP"]
        out[b] = acc_np.T + proj_b
    return out, res


def kernel(x, mask, qkv_w, proj_w, proj_b):
    out, _ = run(x, mask, qkv_w, proj_w, proj_b, trace=False)
    return out


# revision 36
# speedup vs baseline: 1.3836x; 1.0562x over previous
"""Trainium2 Bass kernel for nn_Attention (B=4, N=2048, dim=1024, 16 heads).

Sharding: each of the 8 cores handles one (batch, head-group) pair —
batch b = core//2, head-group g = core%2 (8 heads each). Per core:
  qkv part  : Q^T,K^T = W_{q,k}[g] @ x_b^T (d-major), V natural
              (+ a ones column per head for the softmax1 denominator)
  attention : S^T = K^T-chunk.T @ Q^T-padded (full 128-row contraction;
              each head's Q^T lives in its own 128-row tile with the
              other head's rows zeroed, so every matmul in the kernel
              shares one PE config and stationary reloads stay hidden
              in the background weight buffer),
              P^T = exp(S^T * scale)  (no max subtraction: logits ~ N(0,1)),
              O^T[d,q] (+denom row) = [V|1].T @ P^T  accumulated over k,
              software-pipelined one k-chunk behind the exp stream
  normalize : recip = exp-free 1/(1+denom) on DVE, partition-broadcast
              via DRAM bounce, multiply
  proj      : OUT^T = Wp[g]-slices @ H^T   (partial over head channels)
Host side: per batch, out[b] = (OUT^T_{2b} + OUT^T_{2b+1}).T + proj_b.

All non-score PE work (QKV build, V build, projection) is emitted as
~2-matmul filler pieces spread evenly across the exp-paced units.
"""

import numpy as np
import ml_dtypes
from contextlib import ExitStack

import concourse.bass as bass
import concourse.tile as tile
from concourse import mybir
from concourse.bass_utils import run_bass_kernel_spmd

BF16 = mybir.dt.bfloat16
F32 = mybir.dt.float32
AF = mybir.ActivationFunctionType
NPBF16 = ml_dtypes.bfloat16

N_CORES = 8
B = 4
N = 2048          # tokens per batch
C = 1024          # model dim
NH = 8            # heads per core
HD = 64           # head dim
DQ = NH * HD      # q/k/v dims per core (512)
SCALE = HD ** -0.5
CC = C // 128     # contraction chunks (8)
QB = N // 512     # q blocks of 512 (4)
KC = N // 128     # k chunks of 128 (16)
HC = DQ // 128    # head pairs (4)
OT = C // 128     # output row tiles (8)
_MAX_WAITS = 1


def _split_excess_waits(nc):
    """This walrus build rejects >1 semaphore wait per instruction
    ("Too many sync wait commands"); move the excess onto NOPs inserted
    immediately before the offending instruction on the same engine."""
    n_new = 0
    for f in nc.m.functions:
        for bb in f.blocks:
            insts = bb.instructions
            i = 0
            while i < len(insts):
                inst = insts[i]
                si = inst.sync_info
                if si is not None and si.on_wait and len(si.on_wait) > _MAX_WAITS:
                    waits = list(si.on_wait)
                    keep, rest = waits[:_MAX_WAITS], waits[_MAX_WAITS:]
                    nops = []
                    while rest:
                        chunk, rest = rest[:_MAX_WAITS], rest[_MAX_WAITS:]
                        nop = mybir.InstNoOp(
                            name=f"wait-split-{n_new}", ins=[], outs=[])
                        n_new += 1
                        nop.engine = inst.engine
                        nop.sync_info = mybir.SyncInfo(on_wait=chunk, on_update=[])
                        nops.append(nop)
                    inst.sync_info = mybir.SyncInfo(
                        on_wait=keep, on_update=list(si.on_update or []))
                    for j, nop in enumerate(nops):
                        insts.insert(i + j, nop)
                    i += len(nops)
                i += 1
    return n_new


def _build(ctx: ExitStack, tc: tile.TileContext, xT, wqT, wkT, wvT, wpT, outP):
    nc = tc.nc

    persist = ctx.enter_context(tc.tile_pool(name="persist", bufs=1))
    p_pool = ctx.enter_context(tc.tile_pool(name="p", bufs=8))
    stg_pool = ctx.enter_context(tc.tile_pool(name="stg", bufs=3))
    den_pool = ctx.enter_context(tc.tile_pool(name="den2", bufs=2))
    dram_pool = ctx.enter_context(tc.tile_pool(name="scr", bufs=1, space="DRAM"))
    acc = ctx.enter_context(tc.tile_pool(name="acc", bufs=2, space="PSUM"))
    opair = ctx.enter_context(tc.tile_pool(name="opair", bufs=2, space="PSUM"))
    sc_pool = ctx.enter_context(tc.tile_pool(name="sc", bufs=2, space="PSUM"))

    wq = persist.tile([128, CC, DQ], BF16, tag="wq")
    wk = persist.tile([128, CC, DQ], BF16, tag="wk")
    wv = persist.tile([128, CC, DQ], BF16, tag="wv")
    wp = persist.tile([128, HC, C], BF16, tag="wp")

    # per-head-padded Q^T tiles: head hp of pair hc occupies rows
    # 64*hp..64*hp+63; the other 64 rows are zero so a full-128-row
    # matmul against the pair's K^T chunk yields that head's scores
    qTs = [[persist.tile([128, N], BF16, tag=f"qT{i}_{hp}",
                         name=f"qT{i}_{hp}") for hp in range(2)]
           for i in range(HC)]
    kTs = [persist.tile([128, N], BF16, tag=f"kT{i}", name=f"kT{i}")
           for i in range(HC)]
    vs = [persist.tile([128, NH * (HD + 1)], BF16, tag=f"v{i}", name=f"v{i}")
          for i in range(KC)]
    hT = persist.tile([128, HC, N], BF16, tag="hT")

    den_drams = [dram_pool.tile([2, N], F32, tag=f"dend{i}", name=f"dend{i}")
                 for i in range(HC)]
    recip_drams = [dram_pool.tile([2, N], F32, tag=f"recd{i}", name=f"recd{i}")
                   for i in range(HC)]

    # PE warmup first: ~4.5us of dummy matmuls so the HAM clock gate
    # opens (K=8/8, 2.4 GHz) before the real QKV matmuls start; without
    # this the whole head phase runs at 1.2 GHz
    warm = persist.tile([128, 512], BF16, tag="warm")
    nc.vector.memset(warm[:], 0.0)
    for i in range(24):
        wps = acc.tile([128, 512], F32, tag="acc", name="warmps")
        nc.tensor.matmul(wps[:], warm[:, 0:128], warm[:], start=True,
                         stop=True)

    # ones columns for the softmax1 denominator: strided memset touching
    # only the 8 ones columns per tile (a full-tile memset would cost
    # ~500ns each and serialize ahead of the first Q/K copies on DVE)
    for v_t in vs:
        nc.vector.memset(
            v_t[:].rearrange("p (h e) -> p h e", e=HD + 1)[:, :, HD:HD + 1],
            1.0)

    def emit_qpad(hc):
        # zero halves of pair hc's padded Q^T tiles (one-time)
        nc.vector.memset(qTs[hc][0][64:128, :], 0.0)
        nc.vector.memset(qTs[hc][1][0:64, :], 0.0)

    emit_qpad(0)

    oT_pool = ctx.enter_context(tc.tile_pool(name="oT", bufs=2))
    xt_pool = ctx.enter_context(tc.tile_pool(name="xt", bufs=1))
    rb_pool = ctx.enter_context(tc.tile_pool(name="rb", bufs=2))
    xts = [xt_pool.tile([128, N], BF16, tag=f"xt{i}", name=f"xt{i}")
           for i in range(CC)]

    # DMA: a single queue moves only ~70 GB/s, so spread the input loads
    # over all three DMA-capable queues — x on Sync + Scalar (the ACT
    # HW-DGE is free until the exp stream begins), weights on the GpSimd
    # software queue. wq, wk and x tb0 gate the first scores.
    xT_r = xT.ap().rearrange("(cc p) t -> p cc t", p=128)
    wq_r = wqT.ap().rearrange("(cc p) d -> p cc d", p=128)
    wk_r = wkT.ap().rearrange("(cc p) d -> p cc d", p=128)
    wv_r = wvT.ap().rearrange("(cc p) d -> p cc d", p=128)
    for cc in range(0, CC, 2):
        nc.gpsimd.dma_start(
            out=wq[:, cc:cc + 2, :], in_=wq_r[:, cc:cc + 2, :])
        nc.gpsimd.dma_start(
            out=wk[:, cc:cc + 2, :], in_=wk_r[:, cc:cc + 2, :])
        nc.sync.dma_start(
            out=xts[cc][:, 0:512], in_=xT_r[:, cc, 0:512])
        nc.scalar.dma_start(
            out=xts[cc + 1][:, 0:512], in_=xT_r[:, cc + 1, 0:512])
    for cc in range(0, CC, 2):
        nc.gpsimd.dma_start(
            out=wv[:, cc:cc + 2, :], in_=wv_r[:, cc:cc + 2, :])
    for tb in range(1, QB):
        for cc in range(CC):
            eng = nc.sync if cc % 2 == 0 else nc.scalar
            eng.dma_start(
                out=xts[cc][:, tb * 512:(tb + 1) * 512],
                in_=xT_r[:, cc, tb * 512:(tb + 1) * 512])
    nc.gpsimd.dma_start(
        out=wp[:], in_=wpT.ap().rearrange("(hc p) o -> p hc o", p=128))

    # ---- filler piece machinery (all matmuls share the 128x128 config,
    # each piece <= ~2 matmuls so no unit overloads the PE) -------------

    def qk_pieces(hc, tbs=None, which=("q", "k")):
        for tb in (range(QB) if tbs is None else tbs):
            for w in which:
                w_sb = wq if w == "q" else wk
                state = {}

                def mk(cc0, w=w, w_sb=w_sb, tb=tb, state=state, hc=hc):
                    def piece():
                        if cc0 == 0:
                            state["ps"] = acc.tile(
                                [128, 512], F32, tag="acc", name="qkps")
                        ps = state["ps"]
                        for cc in (cc0, cc0 + 1):
                            nc.tensor.matmul(
                                ps[:],
                                w_sb[:, cc, hc * 128:(hc + 1) * 128],
                                xts[cc][:, tb * 512:(tb + 1) * 512],
                                start=(cc == 0), stop=(cc == CC - 1))
                        if cc0 == CC - 2:
                            ts = slice(tb * 512, (tb + 1) * 512)
                            if w == "q":
                                nc.vector.tensor_copy(
                                    qTs[hc][0][0:64, ts], ps[0:64, :])
                                nc.vector.tensor_copy(
                                    qTs[hc][1][64:128, ts], ps[64:128, :])
                            else:
                                nc.vector.tensor_copy(kTs[hc][:, ts], ps[:])
                    return piece
                for cc0 in range(0, CC, 2):
                    yield mk(cc0)

    def v_pieces(tci):
        state = {}

        def mk(cc0):
            def piece():
                if cc0 == 0:
                    state["ps"] = acc.tile(
                        [128, 512], F32, tag="acc", name="vps")
                ps = state["ps"]
                for cc in (cc0, cc0 + 1):
                    nc.tensor.matmul(
                        ps[:],
                        xts[cc][:, tci * 128:(tci + 1) * 128],
                        wv[:, cc, :],
                        start=(cc == 0), stop=(cc == CC - 1))
                if cc0 == CC - 2:
                    nc.vector.tensor_copy(
                        vs[tci][:].rearrange(
                            "p (h e) -> p h e", e=HD + 1)[:, :, 0:HD],
                        ps[:].rearrange("p (h e) -> p h e", e=HD))
            return piece
        for cc0 in range(0, CC, 2):
            yield mk(cc0)

    def proj_pieces(tb):
        # full projection for token block tb: the four head-pair partials
        # accumulate in PSUM (4 matmuls), then one copy + one store —
        # quarter the DVE-copy and output-DMA traffic of per-pair partials
        outP_r = outP.ap().rearrange("(ot p) t -> p ot t", p=128)
        for ot in range(OT):
            def piece(ot=ot):
                ps = acc.tile([128, 512], F32, tag="acc", name="prps")
                for hc in range(HC):
                    nc.tensor.matmul(
                        ps[:],
                        wp[:, hc, ot * 128:(ot + 1) * 128],
                        hT[:, hc, tb * 512:(tb + 1) * 512],
                        start=(hc == 0), stop=(hc == HC - 1))
                so = stg_pool.tile([128, 512], F32, tag="stg", name="so")
                nc.vector.tensor_copy(so[:], ps[:])
                nc.sync.dma_start(
                    out=outP_r[:, ot, tb * 512:(tb + 1) * 512], in_=so[:])
            yield piece

    oT_tiles = {}

    def norm_qb(hc, qb):
        # recip = 1/(1+den) on a [128, 8] reshape (DVE reciprocal cost
        # scales with free size, so the narrow layout is ~60x cheaper
        # than on the raw [1, 512] den row), then partition-broadcast.
        # All hops ride the Sync HW queue — the GpSimd software queue
        # adds ~1us latency per hop plus drains.
        qs = slice(qb * 512, (qb + 1) * 512)
        den2 = den_pool.tile([128, 8], F32, tag="den2", name="den2")
        nc.sync.dma_start(
            out=den2[:],
            in_=den_drams[hc][:, qs].rearrange("h (a i) -> h a i", i=8))
        nc.vector.tensor_scalar_add(den2[:], den2[:], 1.0)
        nc.vector.reciprocal(den2[:], den2[:])
        nc.sync.dma_start(
            out=recip_drams[hc][:, qs].rearrange("h (a i) -> h a i", i=8),
            in_=den2[:])
        rb_t = rb_pool.tile([128, 512], F32, tag="rb", name="rb")
        for half in range(2):
            src = recip_drams[hc][half:half + 1, qs].broadcast_to((64, 512))
            nc.sync.dma_start(out=rb_t[half * 64:(half + 1) * 64, :], in_=src)
        nc.vector.tensor_mul(
            hT[:, hc, qs], oT_tiles[hc][:, qs], rb_t[:])

    def emit_attention(hc, unit_fillers, unit_hooks=None):
        """unit_fillers[qb*KC+kc]: filler pieces to run in that unit.
        unit_hooks: optional dict {unit_index: callable} for norm/proj
        staggering of the final pair."""
        oT_t = oT_pool.tile([128, N], F32, tag="oT", name=f"oT{hc}")
        oT_tiles[hc] = oT_t
        vcols = [(2 * hc + hp) * (HD + 1) for hp in range(2)]
        for qb in range(QB):
            qs = slice(qb * 512, (qb + 1) * 512)
            o_ps = [opair.tile([128, 512], F32, tag="opair", name=f"ops{hp}")
                    for hp in range(2)]

            def attn_chunk(kc, p_sb):
                for hp in range(2):
                    nc.tensor.matmul(
                        o_ps[hp][0:HD + 1, :],
                        vs[kc][:, vcols[hp]:vcols[hp] + HD + 1],
                        p_sb[:, hp, :],
                        start=(kc == 0), stop=(kc == KC - 1))

            # software pipeline depth 2: attnV runs two chunks behind the
            # exp stream, so at q-block boundaries the o_ps bank reuse
            # (gated on the previous block's drain copies) never blocks
            # the next scores in the in-order PE queue
            pend = []
            for kc in range(KC):
                u = qb * KC + kc
                if unit_hooks and u in unit_hooks:
                    unit_hooks[u]()
                # scores: full-128-row matmuls against the padded Q^T
                # tiles — same PE config as every other matmul here
                s_ps = sc_pool.tile([128, 2, 512], F32, tag="sc")
                for hp in range(2):
                    nc.tensor.matmul(
                        s_ps[:, hp, :],
                        kTs[hc][:, kc * 128:(kc + 1) * 128],
                        qTs[hc][hp][:, qs],
                        start=True, stop=True)
                p_sb = p_pool.tile([128, 2, 512], BF16, tag="p")
                nc.scalar.activation(
                    out=p_sb[:], in_=s_ps[:], func=AF.Exp, scale=SCALE)
                pend.append((kc, p_sb))
                if len(pend) > 2:
                    attn_chunk(*pend.pop(0))
                for piece in unit_fillers[u]:
                    piece()
            for item in pend:
                attn_chunk(*item)

            # drain O^T + denominator rows (head 0 lands in place; head 1
            # needs a partition shift via DMA). The softmax1 reciprocal
            # 1/(1+den) is computed right here on the staged den row, so
            # norm_qb later is just a broadcast DMA + multiply.
            for hp in range(2):
                stg = stg_pool.tile([128, 512], F32, tag="stg")
                if hp == 0:
                    nc.vector.tensor_copy(oT_t[0:HD, qs], o_ps[0][0:HD, :])
                    nc.vector.tensor_copy(
                        stg[HD:HD + 1, :], o_ps[0][HD:HD + 1, :])
                else:
                    nc.vector.tensor_copy(
                        stg[0:HD + 1, :], o_ps[1][0:HD + 1, :])
                    nc.sync.dma_start(
                        out=oT_t[HD:2 * HD, qs], in_=stg[0:HD, :])
                nc.sync.dma_start(
                    out=den_drams[hc][hp:hp + 1, qs],
                    in_=stg[HD:HD + 1, :])

    # ---- static filler schedule ------------------------------------------
    def spread(units, pieces):
        pieces = list(pieces)
        if not pieces:
            return
        for i, piece in enumerate(pieces):
            units[i * len(units) // len(pieces)].append(piece)

    def unit_lists():
        return [[] for _ in range(QB * KC)]

    # pair 0: qb0 must build V (one chunk per unit, just ahead of its
    # first consumer — attnV runs 2 behind, so chunk kc is due at unit
    # kc+2); each q-block also builds the next q-block's Q tile; qb1-3
    # carry the K/Q build of pair 1 (minus the k-parts deferred to
    # pair 1's own early units)
    uf0 = unit_lists()
    # pair 0's own K tiles for token blocks 1-3 stream in just ahead of
    # the score chunks that consume them (chunk 4j needs block j's K),
    # right behind the corresponding x DMA
    for tb in range(1, QB):
        spread([uf0[u] for u in range((tb - 1) * 4, tb * 4 - 1)],
               qk_pieces(0, tbs=(tb,), which=("k",)))
    for kc in range(KC):
        pieces = list(v_pieces(kc))
        slots = [max(kc - 1, 0), min(kc + 1, KC - 1)]
        for i, piece in enumerate(pieces):
            uf0[slots[i * len(slots) // len(pieces)]].append(piece)
    for tb in range(1, QB):
        spread([uf0[u] for u in range((tb - 1) * KC + 8, tb * KC)],
               qk_pieces(0, tbs=(tb,), which=("q",)))
    spread([uf0[u] for u in range(KC, QB * KC)],
           list(qk_pieces(1, which=("q",)))
           + list(qk_pieces(1, tbs=(0, 1), which=("k",))))
    uf0[KC].append(lambda: emit_qpad(1))

    # pair 1: rest of pair 1's K (its chunks kc>=8 are consumed from
    # unit 8 on), K/Q of pair 2; pair 2: K/Q of pair 3
    uf1 = unit_lists()
    spread([uf1[u] for u in range(0, 8)],
           qk_pieces(1, tbs=(2, 3), which=("k",)))
    spread([uf1[u] for u in range(8, QB * KC, 2)], qk_pieces(2))
    uf1[KC].append(lambda: emit_qpad(2))
    uf2 = unit_lists()
    spread([uf2[u] for u in range(0, QB * KC, 2)], qk_pieces(3))
    uf2[KC].append(lambda: emit_qpad(3))

    # pair 3 carries the (cross-pair accumulated) projection: token
    # block tb is ready once norm(3, tb) ran; its norm fires one unit
    # into the next q-block and the proj pieces trail 7+ units behind
    # so their PSUM->copy->store chains never head-of-line block scores
    uf3 = unit_lists()
    hooks3 = {}
    for qb in range(1, QB):
        hooks3[qb * KC + 1] = (lambda qb=qb: norm_qb(3, qb - 1))
        for i, piece in enumerate(proj_pieces(qb - 1)):
            hooks3[qb * KC + 8 + i] = piece

    # norms for pairs 0-2 run at the start of the NEXT pair's stream
    # (hT[hc] must be ready before pair 3's proj); they cost no PE
    def norm_hooks(hc):
        hooks = {}
        for qb in range(QB):
            def hook(hc=hc, qb=qb):
                norm_qb(hc, qb)
            hooks[qb * 4 + 2] = hook  # early units of the next pass
        return hooks

    # ---- emission ---------------------------------------------------------
    # minimal head: only what the first score chunks need — K and Q of
    # token block 0 (the other K blocks stream as early fillers above)
    for piece in qk_pieces(0, tbs=(0,), which=("k",)):
        piece()
    for piece in qk_pieces(0, tbs=(0,), which=("q",)):
        piece()

    emit_attention(0, uf0)
    emit_attention(1, uf1, unit_hooks=norm_hooks(0))
    emit_attention(2, uf2, unit_hooks=norm_hooks(1))
    emit_attention(3, uf3, unit_hooks={**norm_hooks(2), **hooks3})
    # tail: the last norm's DMA chain takes ~4us of PE idle — enough for
    # the HAM gate to re-throttle the PE to 1.2 GHz. Keep it warm with
    # dummy matmuls so the final projection runs at full clock.
    norm_qb(3, QB - 1)
    for i in range(10):
        wps = acc.tile([128, 512], F32, tag="acc", name="warmps2")
        nc.tensor.matmul(wps[:], warm[:, 0:128], warm[:], start=True,
                         stop=True)
    for piece in proj_pieces(QB - 1):
        piece()


_CACHED = None


def _get_nc():
    global _CACHED
    if _CACHED is None:
        nc = bass.Bass("TRN2", target_bir_lowering=False, debug=False)
        xT = nc.dram_tensor("xT", [C, N], BF16, kind="ExternalInput")
        wqT = nc.dram_tensor("wqT", [C, DQ], BF16, kind="ExternalInput")
        wkT = nc.dram_tensor("wkT", [C, DQ], BF16, kind="ExternalInput")
        wvT = nc.dram_tensor("wvT", [C, DQ], BF16, kind="ExternalInput")
        wpT = nc.dram_tensor("wpT", [DQ, C], BF16, kind="ExternalInput")
        outP = nc.dram_tensor("outP", [C, N], F32, kind="ExternalOutput")
        with tile.TileContext(nc) as tc:
            with ExitStack() as ctx:
                _build(ctx, tc, xT, wqT, wkT, wvT, wpT, outP)
        _split_excess_waits(nc)
        _CACHED = nc
    return _CACHED


def run(x, mask, qkv_w, proj_w, proj_b, trace=False):
    x = np.asarray(x, dtype=np.float32)
    qkv_w = np.asarray(qkv_w, dtype=np.float32)
    proj_w = np.asarray(proj_w, dtype=np.float32)
    proj_b = np.asarray(proj_b, dtype=np.float32)

    in_maps = []
    for core in range(N_CORES):
        b, g = core // 2, core % 2
        r = slice(512 * g, 512 * g + 512)
        in_maps.append({
            "xT": np.ascontiguousarray(x[b].T).astype(NPBF16),
            "wqT": np.ascontiguousarray(qkv_w[r].T).astype(NPBF16),
            "wkT": np.ascontiguousarray(qkv_w[1024:][r].T).astype(NPBF16),
            "wvT": np.ascontiguousarray(qkv_w[2048:][r].T).astype(NPBF16),
            "wpT": np.ascontiguousarray(proj_w[:, r].T).astype(NPBF16),
        })

    nc = _get_nc()
    res = run_bass_kernel_spmd(
        nc, in_maps, core_ids=list(range(N_CORES)), trace=trace)

    out = np.empty((B, N, C), dtype=np.float32)
    for b in range(B):
        acc_np = res.results[2 * b]["outP"] + res.results[2 * b + 1]["outP"]
        out[b] = acc_np.T + proj_b
    return out, res


def kernel(x, mask, qkv_w, proj_w, proj_b):
    out, _ = run(x, mask, qkv_w, proj_w, proj_b, trace=False)
    return out
